# revision 3
# baseline (speedup 1.0000x reference)
# GATNet (2-layer GAT, 8 heads x 8 then 1 head x 40) on 8 trn2 NeuronCores.
#
# Strategy (dst-sharded graph parallel, 3 SPMD launches):
#   P1: per-core node projection of its 1/8 node shard:
#       table1 row  = [h1(64) | al_src1(8) | pad]  (fp16, 128 elems = 256B)
#       al_dst1 row = [al_dst1(8) | pad]           (fp16, 256B)
#       (host concatenates the 8 table1 shards into the full gather table)
#   P2: per-core edge stage for GAT layer 1 over the core's dst-owned edges
#       (dst-sorted, grouped in 128-dst blocks, 4 src-chunk substreams for
#       int16 gather indices). Per 128-edge tile:
#         - dma_gather of table1 rows by src (256B/edge)
#         - al_dst per edge via small one-hot matmul (selT [32,128] x
#           gathered al_dst rows of the tile's <=32 distinct dsts)
#         - e = leaky_relu(al_src + al_dst); ex = exp(min(e, 11)) (staged)
#         - mask[e,d] = (dstoff[e] == iota[d]) built on DVE (fp16)
#         - PSUM accumulate [U|S] = mask.T @ [ex*h | ex] per dst block
#       Block epilogue: out1 = relu(U/S + b1); z = out1 @ [W2|W2 a_src2|W2 a_dst2]
#       -> table2 row = [z(40) | as2 | 1.0 | pad] fp16, al_dst2 rows (outputs).
#   P3: same edge stage for layer 2 (H=1): scaled mask = (dstoff==iota)*ex in
#       one DVE op, U2|S2 = mask_s.T @ [z|as2|1], epilogue log_softmax -> out.
#
# The edge schedule (tiles per (block, chunk)) is shared by all 8 cores
# (max over cores), so one NEFF per launch runs SPMD with per-core inputs.

import math
import os
import numpy as np

N = 100000
E = 3200000
F_IN = 512
H1, C1 = 8, 8
D1 = H1 * C1            # 64
D2 = 40                 # 1 head x 40
NEG_SLOPE = 0.2
CLAMP = 11.0            # exp clamp so fp16 ex stays finite (e>11 has P~1e-8)

NC_CORES = 8
NPER = 12544            # 98 * 128 owned dsts per core (>= ceil(100000/8))
NBLK = NPER // 128      # 98
NPAD = NPER * NC_CORES  # 100352
CHUNKS = 4
CROWS = NPAD // CHUNKS  # 25088 (< 32767 so int16 gather idxs work)
ROW = 128               # fp16 elems per gather-table row (256B)
SEL_K = 32              # max distinct dsts per 128-edge tile
SBB = 4                 # dst blocks per superblock (gather granularity)
A1 = D1 + H1            # 72: [h | al_src]
W2COLS = D2 + 2         # 42: [W2 | W2 a_src2 | W2 a_dst2]

_NC_CACHE = {}
LAST_EXEC_TIMES = []    # test.py reads this (ns per launch, if traced)
LAST_RUNS = []          # (nc, in_maps) per launch, for test.py timing
LAST_TRACES = []        # perfetto trace paths per launch, if traced


# ----------------------------------------------------------------- host prep

def _wrap16(a):
    """Edge-stream array (len % 16 == 0) -> [128, len//16] int16 wrapped
    layout: position i lives at [i % 16, i // 16], replicated to 8x16 rows."""
    a = np.asarray(a, dtype=np.int16)
    w = a.reshape(-1, 16).T  # [16, len//16]
    return np.tile(w, (8, 1)).copy()


def _schedule(counts):
    """counts: [NC, NBLK, CHUNKS] real edge counts. Returns T [NBLK][CHUNKS]
    tiles per (block, chunk), shared across cores."""
    mx = counts.max(axis=0)  # [NBLK, CHUNKS]
    T = np.ceil(mx / 128).astype(np.int64)
    return T


class Sched:
    """Static schedule metadata shared by all cores (drives program build and
    host stream packing identically)."""

    def __init__(self, T, sbb=SBB):
        self.T = T                       # [NBLK][CHUNKS]
        nblk = T.shape[0]
        self.sbs = []                    # list of dicts
        gbase = 0                        # gather stream base, slots/128 units
        mtbase = 0                       # mini/selT stream base, padded tiles
        tpbase = 0                       # processing-order tile base
        for s0 in range(0, nblk, sbb):
            blocks = list(range(s0, min(s0 + sbb, nblk)))
            ntc = [int(T[blocks, c].sum()) for c in range(CHUNKS)]
            gb = []
            for c in range(CHUNKS):
                gb.append(gbase)
                gbase += ntc[c]
            ntiles = sum(ntc)
            binfo = []
            tg = 0
            mg = 0                   # mini/selT sweeps in this sb
            for b in blocks:
                tiles = []
                for c in range(CHUNKS):
                    # slot index of (b,c,t) inside gt[c] for this sb
                    off = int(T[[bb for bb in blocks if bb < b], c].sum())
                    for t in range(int(T[b, c])):
                        tiles.append((c, off + t, tg))
                        tg += 1
                binfo.append((b, tiles, mg))   # mg = block's sweep base in sb
                mg += (len(tiles) + 2) // 3    # 3 tiles per 128-row sweep
            self.sbs.append(dict(
                blocks=binfo, ntc=ntc, gbase=gb, ntiles=ntiles,
                mg=mg, mtbase=mtbase, tpbase=tpbase))
            mtbase += mg
            tpbase += ntiles
        self.ntiles = tpbase             # total (processing-order) tiles
        self.mtiles = mtbase             # total mini/selT sweeps
        self.nslot = int(T.sum()) * 128  # total gather slots
        self.ntmax = max(
            sum(int(T[b, c]) for c in range(CHUNKS))
            for b in range(nblk))        # max tiles per block
        self.ntcmax = max(max(sb["ntc"]) for sb in self.sbs)


def _prep_core(k, src, dst, T, sched):
    """Build one core's edge streams. src/dst: this core's edges (dst local)."""
    blk = dst // 128
    doff = dst % 128
    ch = src // CROWS
    order = np.lexsort((doff, ch, blk))
    src, doff, ch, blk = src[order], doff[order], ch[order], blk[order]
    # group boundaries per (blk, ch)
    nblk = T.shape[0]
    key = blk * CHUNKS + ch
    starts = np.searchsorted(key, np.arange(nblk * CHUNKS))
    ends = np.searchsorted(key, np.arange(nblk * CHUNKS) + 1)

    gidx = np.zeros(sched.nslot, np.int16)
    do16 = np.full((128, sched.ntiles), -1.0, np.float16)
    do32 = np.full((128, sched.ntiles), -1.0, np.float32)
    mini = np.zeros(sched.mtiles * 128, np.int16)
    selg = np.zeros((128, sched.mtiles * 128), np.float16)

    gpos = 0
    for sb in sched.sbs:
        # gather-stream order: (c, b, t); processing order: (b, c, t)
        for c in range(CHUNKS):
            for b, tiles, swb in sb["blocks"]:
                s, e = starts[b * CHUNKS + c], ends[b * CHUNKS + c]
                nsl = int(T[b, c]) * 128
                if nsl == 0:
                    continue
                n = e - s
                gidx[gpos:gpos + n] = (src[s:e] % CROWS).astype(np.int16)
                ctiles = [x for x in tiles if x[0] == c]  # t asc
                tl0 = tiles.index(ctiles[0]) if ctiles else 0
                for t, (_, slot, tg) in enumerate(ctiles):
                    lo, hi = t * 128, min((t + 1) * 128, n)
                    if hi <= lo:
                        continue
                    m = hi - lo
                    seg = doff[s + lo:s + hi]
                    tcol = sb["tpbase"] + tg
                    do16[:m, tcol] = seg.astype(np.float16)
                    do32[:m, tcol] = seg.astype(np.float32)
                    u, ranks = np.unique(seg, return_inverse=True)
                    assert len(u) <= SEL_K, f"{len(u)} distinct dsts in tile"
                    tl = tl0 + t                      # block-local tile index
                    sw = sb["mtbase"] + swb + tl // 3  # global sweep
                    mbase = sw * 128 + (tl % 3) * 32
                    mini[mbase:mbase + len(u)] = (b * 128 + u).astype(np.int16)
                    prow = 32 * (tl % 3) + ranks
                    pcol = 128 * sw + np.arange(m)
                    selg[prow, pcol] = 1.0
                gpos += nsl
    assert gpos == sched.nslot
    return dict(gidx=_wrap16(gidx), mini=_wrap16(mini), selt=selg,
                do16=do16, do32=do32)


def _prepare(x, edge_index, W1, a_src1, a_dst1, W2, a_src2, a_dst2):
    src = np.concatenate([np.asarray(edge_index[0]),
                          np.arange(N, dtype=np.int64)]).astype(np.int64)
    dst = np.concatenate([np.asarray(edge_index[1]),
                          np.arange(N, dtype=np.int64)]).astype(np.int64)
    owner = dst // NPER
    per_core = []
    counts = np.zeros((NC_CORES, NBLK, CHUNKS), np.int64)
    for k in range(NC_CORES):
        m = owner == k
        s_k = src[m]
        d_k = dst[m] - k * NPER
        per_core.append((s_k, d_k))
        b = d_k // 128
        c = s_k // CROWS
        np.add.at(counts[k], (b, c), 1)
    T = _schedule(counts)
    sched = Sched(T)

    streams = [_prep_core(k, s_k, d_k, T, sched)
               for k, (s_k, d_k) in enumerate(per_core)]

    # P1 inputs
    W1f = np.asarray(W1, np.float32)                       # [512, 64]
    W1as = np.stack([W1f[:, h * C1:(h + 1) * C1] @ np.asarray(a_src1)[h]
                     for h in range(H1)], axis=1)          # [512, 8]
    W1ad = np.stack([W1f[:, h * C1:(h + 1) * C1] @ np.asarray(a_dst1)[h]
                     for h in range(H1)], axis=1)
    W1R = np.concatenate([W1f, W1as, W1ad], axis=1)        # [512, 80]
    w1r = W1R.reshape(F_IN // 128, 128, 80).transpose(1, 0, 2).astype(
        np.float16).copy()                                 # [128, KC, 80]

    xf = np.zeros((NPAD, F_IN), np.float32)
    xf[:N] = np.asarray(x, np.float32)
    xT_cores = []
    for k in range(NC_CORES):
        xs = xf[k * NPER:(k + 1) * NPER]                   # [NPER, 512]
        xt = xs.T.reshape(F_IN // 128, 128, NPER).transpose(1, 0, 2)
        xT_cores.append(np.ascontiguousarray(xt, dtype=np.float16))

    # P2 consts
    W2f = np.asarray(W2, np.float32)                       # [64, 40]
    w2r = np.concatenate(
        [W2f, W2f @ np.asarray(a_src2)[0][:, None],
         W2f @ np.asarray(a_dst2)[0][:, None]], axis=1).astype(np.float16)

    return sched, streams, w1r, xT_cores, w2r


# ------------------------------------------------------------- bass builders

def _bass_mods():
    import concourse.bacc as bacc
    import concourse.bass as bass
    import concourse.tile as tile
    import concourse.mybir as mybir
    return bacc, bass, tile, mybir


def build_p1():
    bacc, bass, tile, mybir = _bass_mods()
    from contextlib import ExitStack
    F16, F32 = mybir.dt.float16, mybir.dt.float32
    KC = F_IN // 128

    nc = bacc.Bacc("TRN2", target_bir_lowering=False)
    xT = nc.dram_tensor("xT", [128, KC, NPER], F16, kind="ExternalInput")
    w1r = nc.dram_tensor("w1r", [128, KC, 80], F16, kind="ExternalInput")
    t1 = nc.dram_tensor("t1s", [NPER, ROW], F16, kind="ExternalOutput")
    ad1 = nc.dram_tensor("ad1s", [NPER, ROW], F16, kind="ExternalOutput")
    with tile.TileContext(nc) as tc, ExitStack() as ctx:
        cons = ctx.enter_context(tc.tile_pool(name="cons", bufs=1))
        xp = ctx.enter_context(tc.tile_pool(name="xp", bufs=3))
        rp = ctx.enter_context(tc.tile_pool(name="rp", bufs=4))
        pp = ctx.enter_context(tc.tile_pool(name="pp", bufs=2, space="PSUM"))
        w1 = cons.tile([128, KC, 80], F16)
        nc.sync.dma_start(out=w1[:], in_=w1r[:])
        for b in range(NBLK):
            xt = xp.tile([128, KC, 128], F16)
            nc.sync.dma_start(out=xt[:], in_=xT[:, :, b * 128:(b + 1) * 128])
            ps = pp.tile([128, 80], F32)
            for kc in range(KC):
                nc.tensor.matmul(out=ps[:], lhsT=xt[:, kc, :],
                                 rhs=w1[:, kc, :],
                                 start=(kc == 0), stop=(kc == KC - 1))
            row = rp.tile([128, ROW], F16, tag="row")
            nc.vector.memset(row[:], 0.0)
            nc.scalar.copy(row[:, 0:A1], ps[:, 0:A1])
            nc.sync.dma_start(out=t1[b * 128:(b + 1) * 128, :], in_=row[:])
            row2 = rp.tile([128, ROW], F16, tag="row2")
            nc.vector.memset(row2[:], 0.0)
            nc.scalar.copy(row2[:, 0:H1], ps[:, A1:A1 + H1])
            nc.sync.dma_start(out=ad1[b * 128:(b + 1) * 128, :], in_=row2[:])
    nc.compile()
    return nc


def build_edge_layer(sched, layer):
    """layer 1: GAT1 edge stage + z projection (outputs table2 + al_dst2).
    layer 2: GAT2 edge stage + log_softmax (outputs res [NPER, 40] f32)."""
    bacc, bass, tile, mybir = _bass_mods()
    from contextlib import ExitStack
    F16, F32, I16 = mybir.dt.float16, mybir.dt.float32, mybir.dt.int16
    AF = mybir.AluOpType

    NH = H1 if layer == 1 else 1         # heads
    DF = D1 if layer == 1 else D2        # feature cols in gather row
    UW = A1 if layer == 1 else W2COLS    # psum U width (72 / 42)

    nc = bacc.Bacc("TRN2", target_bir_lowering=False)
    tfull = nc.dram_tensor("tfull", [NPAD, ROW], F16, kind="ExternalInput")
    adt = nc.dram_tensor("adt", [NPER, ROW], F16, kind="ExternalInput")
    gidx_d = nc.dram_tensor("gidx", [128, sched.nslot // 16], I16,
                            kind="ExternalInput")
    mini_d = nc.dram_tensor("mini", [128, sched.mtiles * 8], I16,
                            kind="ExternalInput")
    selt_d = nc.dram_tensor("selt", [128, sched.mtiles * 128], F16,
                            kind="ExternalInput")
    if layer == 1:
        do_d = nc.dram_tensor("do16", [128, sched.ntiles], F16,
                              kind="ExternalInput")
        w2r_d = nc.dram_tensor("w2r", [64, W2COLS], F16, kind="ExternalInput")
        b1_d = nc.dram_tensor("b1v", [1, D1], F32, kind="ExternalInput")
        t2 = nc.dram_tensor("t2s", [NPER, ROW], F16, kind="ExternalOutput")
        ad2 = nc.dram_tensor("ad2s", [NPER, ROW], F16, kind="ExternalOutput")
    else:
        do_d = nc.dram_tensor("do32", [128, sched.ntiles], F32,
                              kind="ExternalInput")
        b2_d = nc.dram_tensor("b2v", [1, D2], F32, kind="ExternalInput")
        res = nc.dram_tensor("res", [NPER, D2], F32, kind="ExternalOutput")

    ntmax = sched.ntmax
    with tile.TileContext(nc) as tc, ExitStack() as ctx:
        cons = ctx.enter_context(tc.tile_pool(name="cons", bufs=1))
        gp = ctx.enter_context(tc.tile_pool(name="gp", bufs=2))
        ip = ctx.enter_context(tc.tile_pool(name="ip", bufs=3))
        mp = ctx.enter_context(tc.tile_pool(name="mp", bufs=2))
        sp = ctx.enter_context(tc.tile_pool(name="sp", bufs=2))
        dop = ctx.enter_context(tc.tile_pool(name="dop", bufs=2))
        mkp = ctx.enter_context(tc.tile_pool(name="mkp", bufs=4))
        rhp = ctx.enter_context(tc.tile_pool(name="rhp", bufs=4))
        stp = ctx.enter_context(tc.tile_pool(name="stp", bufs=2))
        bp = ctx.enter_context(tc.tile_pool(name="bp", bufs=3))
        psA = ctx.enter_context(tc.tile_pool(name="psA", bufs=2, space="PSUM"))
        psU = ctx.enter_context(tc.tile_pool(name="psU", bufs=2, space="PSUM"))
        ps1 = ctx.enter_context(tc.tile_pool(name="ps1", bufs=2, space="PSUM"))

        # constants
        iota16 = cons.tile([128, 2, 128], I16)
        nc.gpsimd.iota(iota16[:], pattern=[[0, 2], [1, 128]], base=0,
                       channel_multiplier=0)
        iotaF = cons.tile([128, 2, 128], F16)
        nc.vector.tensor_copy(out=iotaF[:], in_=iota16[:])
        if layer == 1:
            from concourse.masks import make_identity
            ident = cons.tile([128, 128], F16)
            make_identity(nc, ident[:])
            w2s = cons.tile([64, W2COLS], F16)
            nc.sync.dma_start(out=w2s[:], in_=w2r_d[:])
            bB = cons.tile([128, D1], F32)
            nc.sync.dma_start(out=bB[:], in_=b1_d[0:1, :].to_broadcast(
                [128, D1]))
        else:
            bB = cons.tile([128, D2], F32)
            nc.sync.dma_start(out=bB[:], in_=b2_d[0:1, :].to_broadcast(
                [128, D2]))

        _maxsb = int(os.environ.get("GAT_MAX_SB", "0"))
        sbs_iter = sched.sbs[:_maxsb] if _maxsb else sched.sbs
        for sb in sbs_iter:
            gts = []
            for c in range(CHUNKS):
                ntc = sb["ntc"][c]
                if ntc == 0:
                    gts.append(None)
                    continue
                gi = ip.tile([128, ntc * 8], I16, tag=f"gi{c}")
                nc.sync.dma_start(
                    out=gi[:],
                    in_=gidx_d[:, sb["gbase"][c] * 8:
                               (sb["gbase"][c] + ntc) * 8])
                gt = gp.tile([128, ntc, ROW], F16, tag=f"g{c}")
                nc.gpsimd.dma_gather(
                    gt[:], tfull[c * CROWS:(c + 1) * CROWS, :], gi[:],
                    ntc * 128, ntc * 128, ROW, single_packet=False)
                gts.append(gt)
            # al_dst rows for all tiles of this sb
            mg = sb["mg"]
            mi = ip.tile([128, mg * 8], I16, tag="mi")
            nc.sync.dma_start(
                out=mi[:], in_=mini_d[:, sb["mtbase"] * 8:
                                      (sb["mtbase"] + mg) * 8])
            mrows = mp.tile([128, mg, ROW], F16)
            nc.gpsimd.dma_gather(mrows[:], adt[:], mi[:],
                                 mg * 128, mg * 128, ROW,
                                 single_packet=False)
            selt = sp.tile([128, mg, 128], F16)
            nc.sync.dma_start(
                out=selt[:],
                in_=selt_d[:, sb["mtbase"] * 128:
                           (sb["mtbase"] + mg) * 128].rearrange(
                               "p (g e) -> p g e", e=128))
            do = dop.tile([128, sb["ntiles"]], F16 if layer == 1 else F32)
            nc.sync.dma_start(
                out=do[:], in_=do_d[:, sb["tpbase"]:
                                    sb["tpbase"] + sb["ntiles"]])

            _stage = int(os.environ.get("GAT_STAGE", "5"))
            if _stage < 1:
                continue
            for b, tiles, swb in sb["blocks"]:
                ntb = len(tiles)
                if ntb == 0:
                    continue
                if _stage < 2:
                    continue
                # --- al_dst per edge: one K=128 matmul per 3-tile sweep,
                #     rhs is block-diagonal (selt rows off the one-hot band
                #     are zero, so K=128 contraction stays exact) ---
                adps = psA.tile([128, max(ntmax * NH, 8)], F32)
                nsw = (ntb + 2) // 3
                for sw in range(nsw):
                    nj = min(3, ntb - 3 * sw)
                    grp = swb + sw
                    rbd = rhp.tile([128, 3 * NH], F16, tag="rbd")
                    nc.vector.memset(rbd[:], 0.0)
                    for j in range(nj):
                        nc.gpsimd.tensor_copy(
                            out=rbd[32 * j:32 * j + 32, j * NH:(j + 1) * NH],
                            in_=mrows[32 * j:32 * j + 32, grp, 0:NH])
                    nc.tensor.matmul(
                        out=adps[:, 3 * sw * NH:(3 * sw + nj) * NH],
                        lhsT=selt[:, grp, :], rhs=rbd[:, 0:nj * NH],
                        start=True, stop=True)
                if _stage < 3:
                    continue
                # --- staged e / ex ---
                asf = stp.tile([128, max(ntmax * NH, 8)], F32, tag="asf")
                a3 = asf[:].rearrange("p (t h) -> p t h", h=NH)
                tl0 = 0
                for c in range(CHUNKS):
                    ctiles = [x for x in tiles if x[0] == c]
                    if not ctiles:
                        continue
                    s0 = ctiles[0][1]
                    ncn = len(ctiles)
                    nc.scalar.copy(
                        a3[:, tl0:tl0 + ncn, :],
                        gts[c][:, s0:s0 + ncn, DF:DF + NH])
                    tl0 += ncn
                ef = stp.tile([128, max(ntmax * NH, 8)], F32, tag="ef")
                nc.vector.tensor_add(out=ef[:, 0:ntb * NH],
                                     in0=asf[:, 0:ntb * NH],
                                     in1=adps[:, 0:ntb * NH])
                nc.vector.scalar_tensor_tensor(
                    out=ef[:, 0:ntb * NH], in0=ef[:, 0:ntb * NH],
                    scalar=NEG_SLOPE, in1=ef[:, 0:ntb * NH],
                    op0=AF.mult, op1=AF.max)
                if layer == 1:
                    nc.vector.tensor_scalar(
                        out=ef[:, 0:ntb * NH], in0=ef[:, 0:ntb * NH],
                        scalar1=CLAMP, scalar2=None, op0=AF.min)
                ex = stp.tile([128, max(ntmax * NH, 8)],
                              F16 if layer == 1 else F32, tag="ex")
                nc.scalar.activation(ex[:, 0:ntb * NH], ef[:, 0:ntb * NH],
                                     mybir.ActivationFunctionType.Exp)

                if _stage < 4:
                    continue
                # --- masks + weighted aggregation ---
                ups = psU.tile([128, UW], F32)
                if layer == 1:
                    # process tiles in pairs within each chunk run
                    tl = 0
                    for c in range(CHUNKS):
                        ctiles = [x for x in tiles if x[0] == c]
                        i = 0
                        while i < len(ctiles):
                            npair = 2 if i + 1 < len(ctiles) else 1
                            c0, s0, tg0 = ctiles[i]
                            tcol = tg0
                            mk = mkp.tile([128, 2, 128], F16)
                            nc.vector.tensor_tensor(
                                out=mk[:, 0:npair, :],
                                in0=iotaF[:, 0:npair, :],
                                in1=do[:, tcol:tcol + npair].unsqueeze(
                                    2).to_broadcast([128, npair, 128]),
                                op=AF.is_equal)
                            rhs = rhp.tile([128, 2, A1], F16)
                            g4 = gts[c][:, s0:s0 + npair, 0:D1].rearrange(
                                "p t (h c2) -> p t h c2", h=H1)
                            e4 = ex[:, tl * H1:(tl + npair) * H1].rearrange(
                                "p (t h) -> p t h", h=H1).unsqueeze(
                                    3).to_broadcast([128, npair, H1, C1])
                            nc.vector.tensor_tensor(
                                out=rhs[:, 0:npair, 0:D1].rearrange(
                                    "p t (h c2) -> p t h c2", h=H1),
                                in0=g4, in1=e4, op=AF.mult)
                            nc.gpsimd.tensor_copy(
                                out=rhs[:, 0:npair, D1:A1],
                                in_=ex[:, tl * H1:(tl + npair) * H1]
                                .rearrange("p (t h) -> p t h", h=NH))
                            for j in range(npair):
                                nc.tensor.matmul(
                                    out=ups[:], lhsT=mk[:, j, :],
                                    rhs=rhs[:, j, :],
                                    start=(tl + j == 0),
                                    stop=(tl + j == ntb - 1))
                            tl += npair
                            i += npair
                else:
                    for tl, (c, slot, tg) in enumerate(tiles):
                        tcol = tg
                        mk = mkp.tile([128, 128], F16)
                        nc.vector.tensor_scalar(
                            out=mk[:], in0=iotaF[:, 0, :],
                            scalar1=do[:, tcol:tcol + 1],
                            scalar2=ex[:, tl:tl + 1],
                            op0=AF.is_equal, op1=AF.mult)
                        nc.tensor.matmul(
                            out=ups[:], lhsT=mk[:],
                            rhs=gts[c][:, slot, 0:W2COLS],
                            start=(tl == 0), stop=(tl == ntb - 1))

                if _stage < 5:
                    continue
                # --- block epilogue ---
                if layer == 1:
                    seps = bp.tile([128, H1], F32, tag="seps")
                    nc.vector.tensor_scalar(
                        out=seps[:], in0=ups[:, D1:A1], scalar1=1e-16,
                        scalar2=None, op0=AF.add)
                    sinv = bp.tile([128, H1], F32, tag="sinv")
                    nc.vector.reciprocal(sinv[:], seps[:])
                    o1 = bp.tile([128, D1], F32, tag="o1")
                    nc.vector.tensor_tensor(
                        out=o1[:].rearrange("p (h c2) -> p h c2", h=H1),
                        in0=ups[:, 0:D1].rearrange(
                            "p (h c2) -> p h c2", h=H1),
                        in1=sinv[:].unsqueeze(2).to_broadcast(
                            [128, H1, C1]),
                        op=AF.mult)
                    nc.vector.tensor_add(out=o1[:], in0=o1[:], in1=bB[:])
                    o1f = bp.tile([128, D1], F16, tag="o1f")
                    nc.scalar.activation(o1f[:], o1[:],
                                         mybir.ActivationFunctionType.Relu)
                    o1tp = ps1.tile([64, 128], F16, tag="o1tp")
                    nc.tensor.transpose(out=o1tp[:], in_=o1f[:],
                                        identity=ident[:])
                    o1t = bp.tile([64, 128], F16, tag="o1t")
                    nc.vector.tensor_copy(out=o1t[:], in_=o1tp[:])
                    zps = ps1.tile([128, W2COLS], F32, tag="zps")
                    nc.tensor.matmul(out=zps[:], lhsT=o1t[:], rhs=w2s[:],
                                     start=True, stop=True)
                    row = bp.tile([128, ROW], F16, tag="row")
                    nc.vector.memset(row[:], 0.0)
                    nc.scalar.copy(row[:, 0:D2 + 1], zps[:, 0:D2 + 1])
                    nc.vector.memset(row[:, D2 + 1:D2 + 2], 1.0)
                    nc.sync.dma_start(out=t2[b * 128:(b + 1) * 128, :],
                                      in_=row[:])
                    row2 = bp.tile([128, ROW], F16, tag="row2")
                    nc.vector.memset(row2[:], 0.0)
                    nc.scalar.copy(row2[:, 0:1], zps[:, D2 + 1:D2 + 2])
                    nc.sync.dma_start(out=ad2[b * 128:(b + 1) * 128, :],
                                      in_=row2[:])
                else:
                    seps = bp.tile([128, 1], F32, tag="seps")
                    nc.vector.tensor_scalar(
                        out=seps[:], in0=ups[:, D2 + 1:D2 + 2], scalar1=1e-16,
                        scalar2=None, op0=AF.add)
                    sinv = bp.tile([128, 1], F32, tag="sinv")
                    nc.vector.reciprocal(sinv[:], seps[:])
                    o = bp.tile([128, D2], F32, tag="o")
                    nc.vector.tensor_tensor(
                        out=o[:], in0=ups[:, 0:D2],
                        in1=sinv[:].to_broadcast([128, D2]), op=AF.mult)
                    nc.vector.tensor_add(out=o[:], in0=o[:], in1=bB[:])
                    mx = bp.tile([128, 1], F32, tag="mx")
                    nc.vector.reduce_max(out=mx[:], in_=o[:],
                                         axis=mybir.AxisListType.X)
                    om = bp.tile([128, D2], F32, tag="om")
                    nc.vector.tensor_scalar(
                        out=om[:], in0=o[:], scalar1=mx[:, 0:1],
                        scalar2=None, op0=AF.subtract)
                    scr = bp.tile([128, D2], F32, tag="scr")
                    sm = bp.tile([128, 1], F32, tag="sm")
                    nc.scalar.activation(scr[:], om[:],
                                         mybir.ActivationFunctionType.Exp,
                                         accum_out=sm[:])
                    lnt = bp.tile([128, 1], F32, tag="lnt")
                    nc.scalar.activation(lnt[:], sm[:],
                                         mybir.ActivationFunctionType.Ln)
                    ot = bp.tile([128, D2], F32, tag="ot")
                    nc.vector.tensor_scalar(
                        out=ot[:], in0=om[:], scalar1=lnt[:, 0:1],
                        scalar2=None, op0=AF.subtract)
                    nc.sync.dma_start(out=res[b * 128:(b + 1) * 128, :],
                                      in_=ot[:])
    nc.compile()
    return nc


# ------------------------------------------------------------------- runner

def _run(nc, in_maps, trace):
    from concourse.bass_utils import run_bass_kernel_spmd
    LAST_RUNS.append((nc, in_maps))
    try:
        r = run_bass_kernel_spmd(nc, in_maps, core_ids=list(range(NC_CORES)),
                                 trace=trace)
    except ModuleNotFoundError:
        r = run_bass_kernel_spmd(nc, in_maps, core_ids=list(range(NC_CORES)),
                                 trace=False)
    if r.exec_time_ns is not None:
        LAST_EXEC_TIMES.append(r.exec_time_ns)
    if r.instructions_and_trace is not None:
        LAST_TRACES.append(r.instructions_and_trace[1])
    return r.results


def kernel(x, edge_index, W1, a_src1, a_dst1, b1, W2, a_src2, a_dst2, b2):
    trace = bool(int(os.environ.get("GAT_TRACE", "0")))
    LAST_EXEC_TIMES.clear()
    LAST_RUNS.clear()

    sched, streams, w1r, xT_cores, w2r = _prepare(
        x, edge_index, W1, a_src1, a_dst1, W2, a_src2, a_dst2)

    # ---- P1
    nc1 = build_p1()
    in1 = [dict(xT=xT_cores[k], w1r=w1r) for k in range(NC_CORES)]
    r1 = _run(nc1, in1, trace)
    t1_full = np.concatenate([r1[k]["t1s"] for k in range(NC_CORES)], axis=0)
    ad1s = [r1[k]["ad1s"] for k in range(NC_CORES)]

    # ---- P2
    b1v = np.asarray(b1, np.float32).reshape(1, D1)
    nc2 = build_edge_layer(sched, 1)
    in2 = [dict(tfull=t1_full, adt=ad1s[k], gidx=streams[k]["gidx"],
                mini=streams[k]["mini"], selt=streams[k]["selt"],
                do16=streams[k]["do16"], w2r=w2r, b1v=b1v)
           for k in range(NC_CORES)]
    r2 = _run(nc2, in2, trace)
    t2_full = np.concatenate([r2[k]["t2s"] for k in range(NC_CORES)], axis=0)
    ad2s = [r2[k]["ad2s"] for k in range(NC_CORES)]

    # ---- P3
    b2v = np.asarray(b2, np.float32).reshape(1, D2)
    nc3 = build_edge_layer(sched, 2)
    in3 = [dict(tfull=t2_full, adt=ad2s[k], gidx=streams[k]["gidx"],
                mini=streams[k]["mini"], selt=streams[k]["selt"],
                do32=streams[k]["do32"], b2v=b2v)
           for k in range(NC_CORES)]
    r3 = _run(nc3, in3, trace)
    out = np.concatenate([r3[k]["res"] for k in range(NC_CORES)], axis=0)
    return out[:N].astype(np.float32)



# revision 7
# speedup vs baseline: 1.4781x; 1.4781x over previous
# GATNet (2-layer GAT, 8 heads x 8 then 1 head x 40) on 8 trn2 NeuronCores.
#
# Strategy (dst-sharded graph parallel, 3 SPMD launches):
#   P1: per-core node projection of its 1/8 node shard:
#       table1 row  = [h1(64) | al_src1(8) | pad]  (fp16, 128 elems = 256B)
#       ad1 row     = [al_dst1(8)]                 (fp16, 16B)
#       (host concatenates the 8 table1 shards into the full gather table)
#   P2: per-core edge stage for GAT layer 1 over the core's dst-owned edges
#       (dst-sorted, grouped in 128-dst blocks, 4 src-chunk substreams for
#       int16 gather indices). Per 128-edge tile:
#         - dma_gather of table1 rows by src (256B/edge) [SWDGE, GpSimd]
#         - al_dst per edge via transposed one-hot mask: mkT[d,e] =
#           (doT[e] == p_iota[d]) built in one DVE op per block, then a
#           K=128 matmul vs the block's local ad rows (adtAll[:, b, :])
#         - e = leaky_relu(al_src + al_dst); ex = exp(min(e, 11)) (staged)
#         - mask[e,d] = (dstoff[e] == iota[d]) built on DVE, 4 tiles/op
#         - PSUM accumulate [U|S] = mask.T @ [ex*h | ex] per dst block
#       Block epilogue: out1 = relu(U/S + b1); z = out1 @ [W2|W2 a_src2|W2 a_dst2]
#       -> table2 row = [z(40) | as2 | 1.0 | pad] fp16, ad2 [NPER,1] (outputs).
#   P3: same edge stage for layer 2 (H=1): rhs = gathered row * ex (DVE),
#       U2|S2 = mask.T @ rhs, epilogue log_softmax (batched Ln) -> out.
#
# The edge schedule (tiles per (block, chunk)) is shared by all 8 cores
# (max over cores), so one NEFF per launch runs SPMD with per-core inputs.

import math
import os
import numpy as np

N = 100000
E = 3200000
F_IN = 512
H1, C1 = 8, 8
D1 = H1 * C1            # 64
D2 = 40                 # 1 head x 40
NEG_SLOPE = 0.2
CLAMP = 11.0            # exp clamp so fp16 ex stays finite (e>11 has P~1e-8)

NC_CORES = 8
NPER = 12544            # 98 * 128 owned dsts per core (>= ceil(100000/8))
NBLK = NPER // 128      # 98
NPAD = NPER * NC_CORES  # 100352
CHUNKS = 4
CROWS = NPAD // CHUNKS  # 25088 (< 32767 so int16 gather idxs work)
ROW = 128               # fp16 elems per gather-table row (256B)
SBB = 4                 # dst blocks per superblock (gather granularity)
A1 = D1 + H1            # 72: [h | al_src]
W2COLS = D2 + 2         # 42: [W2 | W2 a_src2 | W2 a_dst2]
TB = 4                  # tiles batched per DVE mask/rhs op

_NC_CACHE = {}
LAST_EXEC_TIMES = []    # test.py reads this (ns per launch, if traced)
LAST_RUNS = []          # (nc, in_maps) per launch, for test.py timing
LAST_TRACES = []        # perfetto trace paths per launch, if traced


# ----------------------------------------------------------------- host prep

def _wrap16(a):
    """Edge-stream array (len % 16 == 0) -> [128, len//16] int16 wrapped
    layout: position i lives at [i % 16, i // 16], replicated to 8x16 rows."""
    a = np.asarray(a, dtype=np.int16)
    w = a.reshape(-1, 16).T  # [16, len//16]
    return np.tile(w, (8, 1)).copy()


def _schedule(counts):
    """counts: [NC, NBLK, CHUNKS] real edge counts. Returns T [NBLK][CHUNKS]
    tiles per (block, chunk), shared across cores."""
    mx = counts.max(axis=0)  # [NBLK, CHUNKS]
    T = np.ceil(mx / 128).astype(np.int64)
    return T


class Sched:
    """Static schedule metadata shared by all cores (drives program build and
    host stream packing identically)."""

    def __init__(self, T, sbb=SBB):
        self.T = T                       # [NBLK][CHUNKS]
        nblk = T.shape[0]
        self.sbs = []                    # list of dicts
        gbase = 0                        # gather stream base, slots/128 units
        tpbase = 0                       # processing-order tile base
        for s0 in range(0, nblk, sbb):
            blocks = list(range(s0, min(s0 + sbb, nblk)))
            ntc = [int(T[blocks, c].sum()) for c in range(CHUNKS)]
            gb = []
            for c in range(CHUNKS):
                gb.append(gbase)
                gbase += ntc[c]
            ntiles = sum(ntc)
            binfo = []
            tg = 0
            for b in blocks:
                tiles = []
                for c in range(CHUNKS):
                    # slot index of (b,c,t) inside gt[c] for this sb
                    off = int(T[[bb for bb in blocks if bb < b], c].sum())
                    for t in range(int(T[b, c])):
                        tiles.append((c, off + t, tg))
                        tg += 1
                binfo.append((b, tiles))
            self.sbs.append(dict(
                blocks=binfo, ntc=ntc, gbase=gb, ntiles=ntiles,
                tpbase=tpbase))
            tpbase += ntiles
        self.ntiles = tpbase             # total (processing-order) tiles
        self.nslot = int(T.sum()) * 128  # total gather slots
        self.ntmax = max(
            sum(int(T[b, c]) for c in range(CHUNKS))
            for b in range(nblk))        # max tiles per block
        self.ntcmax = max(max(sb["ntc"]) for sb in self.sbs)


def _prep_core(k, src, dst, T, sched):
    """Build one core's edge streams. src/dst: this core's edges (dst local)."""
    blk = dst // 128
    doff = dst % 128
    ch = src // CROWS
    order = np.lexsort((doff, ch, blk))
    src, doff, ch, blk = src[order], doff[order], ch[order], blk[order]
    # group boundaries per (blk, ch)
    nblk = T.shape[0]
    key = blk * CHUNKS + ch
    starts = np.searchsorted(key, np.arange(nblk * CHUNKS))
    ends = np.searchsorted(key, np.arange(nblk * CHUNKS) + 1)

    gidx = np.zeros(sched.nslot, np.int16)
    do16 = np.full((128, sched.ntiles), -1.0, np.float16)
    do32 = np.full((128, sched.ntiles), -1.0, np.float32)
    doT16 = np.full((1, sched.ntiles * 128), -1.0, np.float16)

    gpos = 0
    for sb in sched.sbs:
        # gather-stream order: (c, b, t); processing order: (b, c, t)
        for c in range(CHUNKS):
            for b, tiles in sb["blocks"]:
                s, e = starts[b * CHUNKS + c], ends[b * CHUNKS + c]
                nsl = int(T[b, c]) * 128
                if nsl == 0:
                    continue
                n = e - s
                gidx[gpos:gpos + n] = (src[s:e] % CROWS).astype(np.int16)
                ctiles = [x for x in tiles if x[0] == c]  # t asc
                for t, (_, slot, tg) in enumerate(ctiles):
                    lo, hi = t * 128, min((t + 1) * 128, n)
                    if hi <= lo:
                        continue
                    m = hi - lo
                    seg = doff[s + lo:s + hi]
                    tcol = sb["tpbase"] + tg
                    do16[:m, tcol] = seg.astype(np.float16)
                    do32[:m, tcol] = seg.astype(np.float32)
                    doT16[0, tcol * 128:tcol * 128 + m] = seg.astype(
                        np.float16)
                gpos += nsl
    assert gpos == sched.nslot
    return dict(gidx=_wrap16(gidx), do16=do16, do32=do32, doT16=doT16)


def _prepare(x, edge_index, W1, a_src1, a_dst1, W2, a_src2, a_dst2):
    src = np.concatenate([np.asarray(edge_index[0]),
                          np.arange(N, dtype=np.int64)]).astype(np.int64)
    dst = np.concatenate([np.asarray(edge_index[1]),
                          np.arange(N, dtype=np.int64)]).astype(np.int64)
    owner = dst // NPER
    per_core = []
    counts = np.zeros((NC_CORES, NBLK, CHUNKS), np.int64)
    for k in range(NC_CORES):
        m = owner == k
        s_k = src[m]
        d_k = dst[m] - k * NPER
        per_core.append((s_k, d_k))
        b = d_k // 128
        c = s_k // CROWS
        np.add.at(counts[k], (b, c), 1)
    T = _schedule(counts)
    sched = Sched(T)

    streams = [_prep_core(k, s_k, d_k, T, sched)
               for k, (s_k, d_k) in enumerate(per_core)]

    # P1 inputs
    W1f = np.asarray(W1, np.float32)                       # [512, 64]
    W1as = np.stack([W1f[:, h * C1:(h + 1) * C1] @ np.asarray(a_src1)[h]
                     for h in range(H1)], axis=1)          # [512, 8]
    W1ad = np.stack([W1f[:, h * C1:(h + 1) * C1] @ np.asarray(a_dst1)[h]
                     for h in range(H1)], axis=1)
    W1R = np.concatenate([W1f, W1as, W1ad], axis=1)        # [512, 80]
    w1r = W1R.reshape(F_IN // 128, 128, 80).transpose(1, 0, 2).astype(
        np.float16).copy()                                 # [128, KC, 80]

    xf = np.zeros((NPAD, F_IN), np.float32)
    xf[:N] = np.asarray(x, np.float32)
    xT_cores = []
    for k in range(NC_CORES):
        xs = xf[k * NPER:(k + 1) * NPER]                   # [NPER, 512]
        xt = xs.T.reshape(F_IN // 128, 128, NPER).transpose(1, 0, 2)
        xT_cores.append(np.ascontiguousarray(xt, dtype=np.float16))

    # P2 consts
    W2f = np.asarray(W2, np.float32)                       # [64, 40]
    w2r = np.concatenate(
        [W2f, W2f @ np.asarray(a_src2)[0][:, None],
         W2f @ np.asarray(a_dst2)[0][:, None]], axis=1).astype(np.float16)

    return sched, streams, w1r, xT_cores, w2r


# ------------------------------------------------------------- bass builders

def _bass_mods():
    import concourse.bacc as bacc
    import concourse.bass as bass
    import concourse.tile as tile
    import concourse.mybir as mybir
    return bacc, bass, tile, mybir


def build_p1():
    bacc, bass, tile, mybir = _bass_mods()
    from contextlib import ExitStack
    F16, F32 = mybir.dt.float16, mybir.dt.float32
    KC = F_IN // 128

    nc = bacc.Bacc("TRN2", target_bir_lowering=False)
    xT = nc.dram_tensor("xT", [128, KC, NPER], F16, kind="ExternalInput")
    w1r = nc.dram_tensor("w1r", [128, KC, 80], F16, kind="ExternalInput")
    t1 = nc.dram_tensor("t1s", [NPER, ROW], F16, kind="ExternalOutput")
    ad1 = nc.dram_tensor("ad1s", [NPER, H1], F16, kind="ExternalOutput")
    with tile.TileContext(nc) as tc, ExitStack() as ctx:
        cons = ctx.enter_context(tc.tile_pool(name="cons", bufs=1))
        xp = ctx.enter_context(tc.tile_pool(name="xp", bufs=3))
        rp = ctx.enter_context(tc.tile_pool(name="rp", bufs=4))
        pp = ctx.enter_context(tc.tile_pool(name="pp", bufs=2, space="PSUM"))
        w1 = cons.tile([128, KC, 80], F16)
        nc.sync.dma_start(out=w1[:], in_=w1r[:])
        for b in range(NBLK):
            xt = xp.tile([128, KC, 128], F16)
            nc.sync.dma_start(out=xt[:], in_=xT[:, :, b * 128:(b + 1) * 128])
            ps = pp.tile([128, 80], F32)
            for kc in range(KC):
                nc.tensor.matmul(out=ps[:], lhsT=xt[:, kc, :],
                                 rhs=w1[:, kc, :],
                                 start=(kc == 0), stop=(kc == KC - 1))
            row = rp.tile([128, ROW], F16, tag="row")
            nc.vector.memset(row[:], 0.0)
            nc.scalar.copy(row[:, 0:A1], ps[:, 0:A1])
            nc.sync.dma_start(out=t1[b * 128:(b + 1) * 128, :], in_=row[:])
            row2 = rp.tile([128, H1], F16, tag="row2")
            nc.scalar.copy(row2[:], ps[:, A1:A1 + H1])
            nc.sync.dma_start(out=ad1[b * 128:(b + 1) * 128, :], in_=row2[:])
    nc.compile()
    return nc


def build_edge_layer(sched, layer):
    """layer 1: GAT1 edge stage + z projection (outputs table2 + ad2).
    layer 2: GAT2 edge stage + log_softmax (outputs res [NPER, 40] f32)."""
    bacc, bass, tile, mybir = _bass_mods()
    from contextlib import ExitStack
    F16, F32, I16 = mybir.dt.float16, mybir.dt.float32, mybir.dt.int16
    AF = mybir.AluOpType

    NH = H1 if layer == 1 else 1         # heads
    DF = D1 if layer == 1 else D2        # feature cols in gather row
    UW = A1 if layer == 1 else W2COLS    # psum U width (72 / 42+1?)

    nc = bacc.Bacc("TRN2", target_bir_lowering=False)
    tfull = nc.dram_tensor("tfull", [NPAD, ROW], F16, kind="ExternalInput")
    adt = nc.dram_tensor("adt", [NPER, NH], F16, kind="ExternalInput")
    gidx_d = nc.dram_tensor("gidx", [128, sched.nslot // 16], I16,
                            kind="ExternalInput")
    doT_d = nc.dram_tensor("doT", [1, sched.ntiles * 128], F16,
                           kind="ExternalInput")
    if layer == 1:
        do_d = nc.dram_tensor("do16", [128, sched.ntiles], F16,
                              kind="ExternalInput")
        w2r_d = nc.dram_tensor("w2r", [64, W2COLS], F16, kind="ExternalInput")
        b1_d = nc.dram_tensor("b1v", [1, D1], F32, kind="ExternalInput")
        t2 = nc.dram_tensor("t2s", [NPER, ROW], F16, kind="ExternalOutput")
        ad2 = nc.dram_tensor("ad2s", [NPER, 1], F16, kind="ExternalOutput")
    else:
        do_d = nc.dram_tensor("do16", [128, sched.ntiles], F16,
                              kind="ExternalInput")
        b2_d = nc.dram_tensor("b2v", [1, D2], F32, kind="ExternalInput")
        res = nc.dram_tensor("res", [NPER, D2], F32, kind="ExternalOutput")

    ntmax = sched.ntmax
    with tile.TileContext(nc) as tc, ExitStack() as ctx:
        cons = ctx.enter_context(tc.tile_pool(name="cons", bufs=1))
        gp = ctx.enter_context(tc.tile_pool(name="gp", bufs=2))
        ip = ctx.enter_context(tc.tile_pool(name="ip", bufs=3))
        dop = ctx.enter_context(tc.tile_pool(name="dop", bufs=2))
        dtp = ctx.enter_context(tc.tile_pool(name="dtp", bufs=2))
        mtp = ctx.enter_context(tc.tile_pool(name="mtp", bufs=2))
        mkp = ctx.enter_context(tc.tile_pool(name="mkp", bufs=4))
        rhp = ctx.enter_context(tc.tile_pool(name="rhp", bufs=4))
        stp = ctx.enter_context(tc.tile_pool(name="stp", bufs=2))
        bp = ctx.enter_context(tc.tile_pool(name="bp", bufs=3))
        psA = ctx.enter_context(tc.tile_pool(name="psA", bufs=2, space="PSUM"))
        psU = ctx.enter_context(tc.tile_pool(name="psU", bufs=2, space="PSUM"))
        ps1 = ctx.enter_context(tc.tile_pool(name="ps1", bufs=2, space="PSUM"))

        # constants
        iota16 = cons.tile([128, TB, 128], I16)
        nc.gpsimd.iota(iota16[:], pattern=[[0, TB], [1, 128]], base=0,
                       channel_multiplier=0)
        iotaF = cons.tile([128, TB, 128], F16)
        nc.vector.tensor_copy(out=iotaF[:], in_=iota16[:])
        iotaP16 = cons.tile([128, 1], I16)
        nc.gpsimd.iota(iotaP16[:], pattern=[[0, 1]], base=0,
                       channel_multiplier=1)
        iotaP = cons.tile([128, 1], F16)
        nc.vector.tensor_copy(out=iotaP[:], in_=iotaP16[:])
        # all al_dst rows of this core: [128, NBLK, NH]
        adtAll = cons.tile([128, NBLK, NH], F16)
        nc.sync.dma_start(
            out=adtAll[:],
            in_=adt[:].rearrange("(b p) h -> p b h", p=128))
        if layer == 1:
            from concourse.masks import make_identity
            ident = cons.tile([128, 128], F16)
            make_identity(nc, ident[:])
            w2s = cons.tile([64, W2COLS], F16)
            nc.sync.dma_start(out=w2s[:], in_=w2r_d[:])
            bB = cons.tile([128, D1], F32)
            nc.sync.dma_start(out=bB[:], in_=b1_d[0:1, :].to_broadcast(
                [128, D1]))
        else:
            bB = cons.tile([128, D2], F32)
            nc.sync.dma_start(out=bB[:], in_=b2_d[0:1, :].to_broadcast(
                [128, D2]))
            omAll = cons.tile([128, NBLK, D2], F32)   # staged for final pass
            smAll = cons.tile([128, NBLK], F32)

        _maxsb = int(os.environ.get("GAT_MAX_SB", "0"))
        sbs_iter = sched.sbs[:_maxsb] if _maxsb else sched.sbs
        for sb in sbs_iter:
            gts = []
            for c in range(CHUNKS):
                ntc = sb["ntc"][c]
                if ntc == 0:
                    gts.append(None)
                    continue
                gi = ip.tile([128, ntc * 8], I16, tag=f"gi{c}")
                nc.sync.dma_start(
                    out=gi[:],
                    in_=gidx_d[:, sb["gbase"][c] * 8:
                               (sb["gbase"][c] + ntc) * 8])
                gt = gp.tile([128, ntc, ROW], F16, tag=f"g{c}")
                nc.gpsimd.dma_gather(
                    gt[:], tfull[c * CROWS:(c + 1) * CROWS, :], gi[:],
                    ntc * 128, ntc * 128, ROW, single_packet=False)
                gts.append(gt)
            do = dop.tile([128, sb["ntiles"]], F16)
            nc.sync.dma_start(
                out=do[:], in_=do_d[:, sb["tpbase"]:
                                    sb["tpbase"] + sb["ntiles"]])

            _stage = int(os.environ.get("GAT_STAGE", "5"))
            if _stage < 1:
                continue
            for b, tiles in sb["blocks"]:
                ntb = len(tiles)
                if ntb == 0:
                    continue
                if _stage < 2:
                    continue
                # --- al_dst per edge: transposed one-hot mask x local ad
                #     rows.  mkT[d, e] = (doT[e] == d); one DVE op per block,
                #     one K=128 matmul (N=NH) per tile. ---
                t0g = sb["tpbase"] + tiles[0][2]     # global first tile
                doTb = dtp.tile([128, ntb * 128], F16)
                nc.sync.dma_start(
                    out=doTb[:],
                    in_=doT_d[0:1, t0g * 128:(t0g + ntb) * 128].to_broadcast(
                        [128, ntb * 128]))
                mkT = mtp.tile([128, ntb * 128], F16)
                nc.vector.tensor_tensor(
                    out=mkT[:], in0=doTb[:],
                    in1=iotaP[:, 0:1].to_broadcast([128, ntb * 128]),
                    op=AF.is_equal)
                adps = psA.tile([128, max(ntmax * NH, 8)], F32)
                for tl in range(ntb):
                    nc.tensor.matmul(
                        out=adps[:, tl * NH:(tl + 1) * NH],
                        lhsT=mkT[:, tl * 128:(tl + 1) * 128],
                        rhs=adtAll[:, b, :],
                        start=True, stop=True)
                if _stage < 3:
                    continue
                # --- staged e / ex ---
                asf = stp.tile([128, max(ntmax * NH, 8)], F32, tag="asf")
                a3 = asf[:].rearrange("p (t h) -> p t h", h=NH)
                tl0 = 0
                for c in range(CHUNKS):
                    ctiles = [x for x in tiles if x[0] == c]
                    if not ctiles:
                        continue
                    s0 = ctiles[0][1]
                    ncn = len(ctiles)
                    nc.scalar.copy(
                        a3[:, tl0:tl0 + ncn, :],
                        gts[c][:, s0:s0 + ncn, DF:DF + NH])
                    tl0 += ncn
                ef = stp.tile([128, max(ntmax * NH, 8)], F32, tag="ef")
                nc.vector.tensor_add(out=ef[:, 0:ntb * NH],
                                     in0=asf[:, 0:ntb * NH],
                                     in1=adps[:, 0:ntb * NH])
                nc.vector.scalar_tensor_tensor(
                    out=ef[:, 0:ntb * NH], in0=ef[:, 0:ntb * NH],
                    scalar=NEG_SLOPE, in1=ef[:, 0:ntb * NH],
                    op0=AF.mult, op1=AF.max)
                nc.vector.tensor_scalar(
                    out=ef[:, 0:ntb * NH], in0=ef[:, 0:ntb * NH],
                    scalar1=CLAMP, scalar2=None, op0=AF.min)
                ex = stp.tile([128, max(ntmax * NH, 8)], F16, tag="ex")
                nc.scalar.activation(ex[:, 0:ntb * NH], ef[:, 0:ntb * NH],
                                     mybir.ActivationFunctionType.Exp)

                if _stage < 4:
                    continue
                # --- masks + weighted aggregation (batch TB tiles per DVE
                #     op; one matmul per tile accumulating into U|S) ---
                ups = psU.tile([128, UW], F32)
                tl = 0
                for c in range(CHUNKS):
                    ctiles = [x for x in tiles if x[0] == c]
                    i = 0
                    while i < len(ctiles):
                        nb = min(TB, len(ctiles) - i)
                        c0, s0, tg0 = ctiles[i]
                        tcol = tg0
                        mk = mkp.tile([128, TB, 128], F16)
                        nc.vector.tensor_tensor(
                            out=mk[:, 0:nb, :],
                            in0=iotaF[:, 0:nb, :],
                            in1=do[:, tcol:tcol + nb].unsqueeze(
                                2).to_broadcast([128, nb, 128]),
                            op=AF.is_equal)
                        rhs = rhp.tile([128, TB, UW], F16)
                        if layer == 1:
                            g4 = gts[c][:, s0:s0 + nb, 0:D1].rearrange(
                                "p t (h c2) -> p t h c2", h=H1)
                            e4 = ex[:, tl * H1:(tl + nb) * H1].rearrange(
                                "p (t h) -> p t h", h=H1).unsqueeze(
                                    3).to_broadcast([128, nb, H1, C1])
                            nc.vector.tensor_tensor(
                                out=rhs[:, 0:nb, 0:D1].rearrange(
                                    "p t (h c2) -> p t h c2", h=H1),
                                in0=g4, in1=e4, op=AF.mult)
                            nc.scalar.copy(
                                out=rhs[:, 0:nb, D1:A1],
                                in_=ex[:, tl * H1:(tl + nb) * H1]
                                .rearrange("p (t h) -> p t h", h=NH))
                        else:
                            e4 = ex[:, tl:tl + nb].unsqueeze(
                                2).to_broadcast([128, nb, W2COLS])
                            nc.vector.tensor_tensor(
                                out=rhs[:, 0:nb, :],
                                in0=gts[c][:, s0:s0 + nb, 0:W2COLS],
                                in1=e4, op=AF.mult)
                        for j in range(nb):
                            nc.tensor.matmul(
                                out=ups[:], lhsT=mk[:, j, :],
                                rhs=rhs[:, j, :],
                                start=(tl + j == 0),
                                stop=(tl + j == ntb - 1))
                        tl += nb
                        i += nb

                if _stage < 5:
                    continue
                # --- block epilogue ---
                if layer == 1:
                    seps = bp.tile([128, H1], F32, tag="seps")
                    nc.vector.tensor_scalar(
                        out=seps[:], in0=ups[:, D1:A1], scalar1=1e-16,
                        scalar2=None, op0=AF.add)
                    sinv = bp.tile([128, H1], F32, tag="sinv")
                    nc.vector.reciprocal(sinv[:], seps[:])
                    o1 = bp.tile([128, D1], F32, tag="o1")
                    nc.vector.tensor_tensor(
                        out=o1[:].rearrange("p (h c2) -> p h c2", h=H1),
                        in0=ups[:, 0:D1].rearrange(
                            "p (h c2) -> p h c2", h=H1),
                        in1=sinv[:].unsqueeze(2).to_broadcast(
                            [128, H1, C1]),
                        op=AF.mult)
                    nc.vector.tensor_add(out=o1[:], in0=o1[:], in1=bB[:])
                    o1f = bp.tile([128, D1], F16, tag="o1f")
                    nc.scalar.activation(o1f[:], o1[:],
                                         mybir.ActivationFunctionType.Relu)
                    o1tp = ps1.tile([64, 128], F16, tag="o1tp")
                    nc.tensor.transpose(out=o1tp[:], in_=o1f[:],
                                        identity=ident[:])
                    o1t = bp.tile([64, 128], F16, tag="o1t")
                    nc.vector.tensor_copy(out=o1t[:], in_=o1tp[:])
                    zps = ps1.tile([128, W2COLS], F32, tag="zps")
                    nc.tensor.matmul(out=zps[:], lhsT=o1t[:], rhs=w2s[:],
                                     start=True, stop=True)
                    row = bp.tile([128, ROW], F16, tag="row")
                    nc.vector.memset(row[:], 0.0)
                    nc.scalar.copy(row[:, 0:D2 + 1], zps[:, 0:D2 + 1])
                    nc.vector.memset(row[:, D2 + 1:D2 + 2], 1.0)
                    nc.sync.dma_start(out=t2[b * 128:(b + 1) * 128, :],
                                      in_=row[:])
                    row2 = bp.tile([128, 1], F16, tag="row2")
                    nc.scalar.copy(row2[:], zps[:, D2 + 1:D2 + 2])
                    nc.sync.dma_start(out=ad2[b * 128:(b + 1) * 128, :],
                                      in_=row2[:])
                else:
                    seps = bp.tile([128, 1], F32, tag="seps")
                    nc.vector.tensor_scalar(
                        out=seps[:], in0=ups[:, D2 + 1:D2 + 2], scalar1=1e-16,
                        scalar2=None, op0=AF.add)
                    sinv = bp.tile([128, 1], F32, tag="sinv")
                    nc.vector.reciprocal(sinv[:], seps[:])
                    o = bp.tile([128, D2], F32, tag="o")
                    nc.vector.tensor_tensor(
                        out=o[:], in0=ups[:, 0:D2],
                        in1=sinv[:].to_broadcast([128, D2]), op=AF.mult)
                    nc.vector.tensor_add(out=o[:], in0=o[:], in1=bB[:])
                    mx = bp.tile([128, 1], F32, tag="mx")
                    nc.vector.reduce_max(out=mx[:], in_=o[:],
                                         axis=mybir.AxisListType.X)
                    nc.vector.tensor_tensor(
                        out=omAll[:, b, :], in0=o[:],
                        in1=mx[:, 0:1].to_broadcast([128, D2]),
                        op=AF.subtract)
                    scr = bp.tile([128, D2], F32, tag="scr")
                    nc.scalar.activation(scr[:], omAll[:, b, :],
                                         mybir.ActivationFunctionType.Exp,
                                         accum_out=smAll[:, b:b + 1])
        if layer == 2 and not _maxsb:
            # final pass: one Ln over all blocks, subtract, write out
            lnt = cons.tile([128, NBLK], F32)
            nc.scalar.activation(lnt[:], smAll[:],
                                 mybir.ActivationFunctionType.Ln)
            for b in range(NBLK):
                ot = bp.tile([128, D2], F32, tag="ot")
                nc.vector.tensor_tensor(
                    out=ot[:], in0=omAll[:, b, :],
                    in1=lnt[:, b:b + 1].to_broadcast([128, D2]),
                    op=AF.subtract)
                nc.sync.dma_start(out=res[b * 128:(b + 1) * 128, :],
                                  in_=ot[:])
    nc.compile()
    return nc


# ------------------------------------------------------------------- runner

def _run(nc, in_maps, trace):
    from concourse.bass_utils import run_bass_kernel_spmd
    LAST_RUNS.append((nc, in_maps))
    try:
        r = run_bass_kernel_spmd(nc, in_maps, core_ids=list(range(NC_CORES)),
                                 trace=trace)
    except ModuleNotFoundError:
        r = run_bass_kernel_spmd(nc, in_maps, core_ids=list(range(NC_CORES)),
                                 trace=False)
    if r.exec_time_ns is not None:
        LAST_EXEC_TIMES.append(r.exec_time_ns)
    if r.instructions_and_trace is not None:
        LAST_TRACES.append(r.instructions_and_trace[1])
    return r.results


def kernel(x, edge_index, W1, a_src1, a_dst1, b1, W2, a_src2, a_dst2, b2):
    trace = bool(int(os.environ.get("GAT_TRACE", "0")))
    LAST_EXEC_TIMES.clear()
    LAST_RUNS.clear()
    LAST_TRACES.clear()

    sched, streams, w1r, xT_cores, w2r = _prepare(
        x, edge_index, W1, a_src1, a_dst1, W2, a_src2, a_dst2)

    # ---- P1
    nc1 = build_p1()
    in1 = [dict(xT=xT_cores[k], w1r=w1r) for k in range(NC_CORES)]
    r1 = _run(nc1, in1, trace)
    t1_full = np.concatenate([r1[k]["t1s"] for k in range(NC_CORES)], axis=0)
    ad1s = [r1[k]["ad1s"] for k in range(NC_CORES)]

    # ---- P2
    b1v = np.asarray(b1, np.float32).reshape(1, D1)
    nc2 = build_edge_layer(sched, 1)
    in2 = [dict(tfull=t1_full, adt=ad1s[k], gidx=streams[k]["gidx"],
                doT=streams[k]["doT16"], do16=streams[k]["do16"],
                w2r=w2r, b1v=b1v)
           for k in range(NC_CORES)]
    r2 = _run(nc2, in2, trace)
    t2_full = np.concatenate([r2[k]["t2s"] for k in range(NC_CORES)], axis=0)
    ad2s = [r2[k]["ad2s"] for k in range(NC_CORES)]

    # ---- P3
    b2v = np.asarray(b2, np.float32).reshape(1, D2)
    nc3 = build_edge_layer(sched, 2)
    in3 = [dict(tfull=t2_full, adt=ad2s[k], gidx=streams[k]["gidx"],
                doT=streams[k]["doT16"], do16=streams[k]["do16"], b2v=b2v)
           for k in range(NC_CORES)]
    r3 = _run(nc3, in3, trace)
    out = np.concatenate([r3[k]["res"] for k in range(NC_CORES)], axis=0)
    return out[:N].astype(np.float32)


# revision 10
# speedup vs baseline: 1.5443x; 1.0448x over previous
# GATNet (2-layer GAT, 8 heads x 8 then 1 head x 40) on 8 trn2 NeuronCores.
#
# Strategy (dst-sharded graph parallel, 3 SPMD launches):
#   P1: per-core node projection of its 1/8 node shard:
#       table1 row  = [h1(64) | al_src1(8) | pad]  (fp16, 128 elems = 256B)
#       ad1 row     = [al_dst1(8)]                 (fp16, 16B)
#       (host concatenates the 8 table1 shards into the full gather table)
#   P2: per-core edge stage for GAT layer 1 over the core's dst-owned edges
#       (dst-sorted, grouped in 128-dst blocks, 4 src-chunk substreams for
#       int16 gather indices). Per 128-edge tile:
#         - dma_gather of table1 rows by src (256B/edge) [SWDGE, GpSimd]
#         - al_dst per edge via transposed one-hot mask: mkT[d,e] =
#           (doT[e] == p_iota[d]) built in one DVE op per block, then a
#           K=128 matmul vs the block's local ad rows (adtAll[:, b, :])
#         - e = leaky_relu(al_src + al_dst); ex = exp(min(e, 11)) (staged)
#         - mask[e,d] = (dstoff[e] == iota[d]) built on DVE, 4 tiles/op
#         - PSUM accumulate [U|S] = mask.T @ [ex*h | ex] per dst block
#       Block epilogue: out1 = relu(U/S + b1); z = out1 @ [W2|W2 a_src2|W2 a_dst2]
#       -> table2 row = [z(40) | as2 | 1.0 | pad] fp16, ad2 [NPER,1] (outputs).
#   P3: same edge stage for layer 2 (H=1): rhs = gathered row * ex (DVE),
#       U2|S2 = mask.T @ rhs, epilogue log_softmax (batched Ln) -> out.
#
# The edge schedule (tiles per (block, chunk)) is shared by all 8 cores
# (max over cores), so one NEFF per launch runs SPMD with per-core inputs.

import math
import os
import numpy as np

N = 100000
E = 3200000
F_IN = 512
H1, C1 = 8, 8
D1 = H1 * C1            # 64
D2 = 40                 # 1 head x 40
NEG_SLOPE = 0.2
CLAMP = 11.0            # exp clamp so fp16 ex stays finite (e>11 has P~1e-8)

NC_CORES = 8
NPER = 12544            # 98 * 128 owned dsts per core (>= ceil(100000/8))
NBLK = NPER // 128      # 98
NPAD = NPER * NC_CORES  # 100352
CHUNKS = 4
CROWS = NPAD // CHUNKS  # 25088 (< 32767 so int16 gather idxs work)
ROW = 128               # fp16 elems per gather-table row (256B)
SBB = 4                 # dst blocks per superblock (gather granularity)
A1 = D1 + H1            # 72: [h | al_src]
W2COLS = D2 + 2         # 42: [W2 | W2 a_src2 | W2 a_dst2]
TB = 4                  # tiles batched per DVE mask/rhs op

_NC_CACHE = {}
LAST_EXEC_TIMES = []    # test.py reads this (ns per launch, if traced)
LAST_RUNS = []          # (nc, in_maps) per launch, for test.py timing
LAST_TRACES = []        # perfetto trace paths per launch, if traced


# ----------------------------------------------------------------- host prep

def _wrap16(a):
    """Edge-stream array (len % 16 == 0) -> [128, len//16] int16 wrapped
    layout: position i lives at [i % 16, i // 16], replicated to 8x16 rows."""
    a = np.asarray(a, dtype=np.int16)
    w = a.reshape(-1, 16).T  # [16, len//16]
    return np.tile(w, (8, 1)).copy()


def _schedule(counts):
    """counts: [NC, NBLK, CHUNKS] real edge counts. Returns T [NBLK][CHUNKS]
    tiles per (block, chunk), shared across cores."""
    mx = counts.max(axis=0)  # [NBLK, CHUNKS]
    T = np.ceil(mx / 128).astype(np.int64)
    return T


class Sched:
    """Static schedule metadata shared by all cores (drives program build and
    host stream packing identically)."""

    def __init__(self, T, sbb=SBB):
        self.T = T                       # [NBLK][CHUNKS]
        nblk = T.shape[0]
        self.sbs = []                    # list of dicts
        gbase = 0                        # gather stream base, slots/128 units
        tpbase = 0                       # processing-order tile base
        for s0 in range(0, nblk, sbb):
            blocks = list(range(s0, min(s0 + sbb, nblk)))
            ntc = [int(T[blocks, c].sum()) for c in range(CHUNKS)]
            gb = []
            for c in range(CHUNKS):
                gb.append(gbase)
                gbase += ntc[c]
            ntiles = sum(ntc)
            binfo = []
            tg = 0
            for b in blocks:
                tiles = []
                for c in range(CHUNKS):
                    # slot index of (b,c,t) inside gt[c] for this sb
                    off = int(T[[bb for bb in blocks if bb < b], c].sum())
                    for t in range(int(T[b, c])):
                        tiles.append((c, off + t, tg))
                        tg += 1
                binfo.append((b, tiles))
            self.sbs.append(dict(
                blocks=binfo, ntc=ntc, gbase=gb, ntiles=ntiles,
                tpbase=tpbase))
            tpbase += ntiles
        self.ntiles = tpbase             # total (processing-order) tiles
        self.nslot = int(T.sum()) * 128  # total gather slots
        self.ntmax = max(
            sum(int(T[b, c]) for c in range(CHUNKS))
            for b in range(nblk))        # max tiles per block
        self.ntcmax = max(max(sb["ntc"]) for sb in self.sbs)


def _prep_core(k, src, dst, T, sched):
    """Build one core's edge streams. src/dst: this core's edges (dst local)."""
    blk = dst // 128
    doff = dst % 128
    ch = src // CROWS
    order = np.lexsort((doff, ch, blk))
    src, doff, ch, blk = src[order], doff[order], ch[order], blk[order]
    # group boundaries per (blk, ch)
    nblk = T.shape[0]
    key = blk * CHUNKS + ch
    starts = np.searchsorted(key, np.arange(nblk * CHUNKS))
    ends = np.searchsorted(key, np.arange(nblk * CHUNKS) + 1)

    gidx = np.zeros(sched.nslot, np.int16)
    do16 = np.full((128, sched.ntiles), -1.0, np.float16)
    do32 = np.full((128, sched.ntiles), -1.0, np.float32)
    doT16 = np.full((1, sched.ntiles * 128), -1.0, np.float16)

    gpos = 0
    for sb in sched.sbs:
        # gather-stream order: (c, b, t); processing order: (b, c, t)
        for c in range(CHUNKS):
            for b, tiles in sb["blocks"]:
                s, e = starts[b * CHUNKS + c], ends[b * CHUNKS + c]
                nsl = int(T[b, c]) * 128
                if nsl == 0:
                    continue
                n = e - s
                gidx[gpos:gpos + n] = (src[s:e] % CROWS).astype(np.int16)
                ctiles = [x for x in tiles if x[0] == c]  # t asc
                for t, (_, slot, tg) in enumerate(ctiles):
                    lo, hi = t * 128, min((t + 1) * 128, n)
                    if hi <= lo:
                        continue
                    m = hi - lo
                    seg = doff[s + lo:s + hi]
                    tcol = sb["tpbase"] + tg
                    do16[:m, tcol] = seg.astype(np.float16)
                    do32[:m, tcol] = seg.astype(np.float32)
                    doT16[0, tcol * 128:tcol * 128 + m] = seg.astype(
                        np.float16)
                gpos += nsl
    assert gpos == sched.nslot
    return dict(gidx=_wrap16(gidx), do16=do16, do32=do32, doT16=doT16)


def _prepare(x, edge_index, W1, a_src1, a_dst1, W2, a_src2, a_dst2):
    src = np.concatenate([np.asarray(edge_index[0]),
                          np.arange(N, dtype=np.int64)]).astype(np.int64)
    dst = np.concatenate([np.asarray(edge_index[1]),
                          np.arange(N, dtype=np.int64)]).astype(np.int64)
    owner = dst // NPER
    per_core = []
    counts = np.zeros((NC_CORES, NBLK, CHUNKS), np.int64)
    for k in range(NC_CORES):
        m = owner == k
        s_k = src[m]
        d_k = dst[m] - k * NPER
        per_core.append((s_k, d_k))
        b = d_k // 128
        c = s_k // CROWS
        np.add.at(counts[k], (b, c), 1)
    T = _schedule(counts)
    sched = Sched(T)

    streams = [_prep_core(k, s_k, d_k, T, sched)
               for k, (s_k, d_k) in enumerate(per_core)]

    # P1 inputs
    W1f = np.asarray(W1, np.float32)                       # [512, 64]
    W1as = np.stack([W1f[:, h * C1:(h + 1) * C1] @ np.asarray(a_src1)[h]
                     for h in range(H1)], axis=1)          # [512, 8]
    W1ad = np.stack([W1f[:, h * C1:(h + 1) * C1] @ np.asarray(a_dst1)[h]
                     for h in range(H1)], axis=1)
    W1R = np.concatenate([W1f, W1as, W1ad], axis=1)        # [512, 80]
    w1r = W1R.reshape(F_IN // 128, 128, 80).transpose(1, 0, 2).astype(
        np.float16).copy()                                 # [128, KC, 80]

    xf = np.zeros((NPAD, F_IN), np.float32)
    xf[:N] = np.asarray(x, np.float32)
    xT_cores = []
    for k in range(NC_CORES):
        xs = xf[k * NPER:(k + 1) * NPER]                   # [NPER, 512]
        xt = xs.T.reshape(F_IN // 128, 128, NPER).transpose(1, 0, 2)
        xT_cores.append(np.ascontiguousarray(xt, dtype=np.float16))

    # P2 consts
    W2f = np.asarray(W2, np.float32)                       # [64, 40]
    w2r = np.concatenate(
        [W2f, W2f @ np.asarray(a_src2)[0][:, None],
         W2f @ np.asarray(a_dst2)[0][:, None]], axis=1).astype(np.float16)

    return sched, streams, w1r, xT_cores, w2r


# ------------------------------------------------------------- bass builders

def _bass_mods():
    import concourse.bacc as bacc
    import concourse.bass as bass
    import concourse.tile as tile
    import concourse.mybir as mybir
    return bacc, bass, tile, mybir


def build_p1():
    bacc, bass, tile, mybir = _bass_mods()
    from contextlib import ExitStack
    F16, F32 = mybir.dt.float16, mybir.dt.float32
    KC = F_IN // 128

    nc = bacc.Bacc("TRN2", target_bir_lowering=False)
    xT = nc.dram_tensor("xT", [128, KC, NPER], F16, kind="ExternalInput")
    w1r = nc.dram_tensor("w1r", [128, KC, 80], F16, kind="ExternalInput")
    t1 = nc.dram_tensor("t1s", [NPER, ROW], F16, kind="ExternalOutput")
    ad1 = nc.dram_tensor("ad1s", [NPER, H1], F16, kind="ExternalOutput")
    with tile.TileContext(nc) as tc, ExitStack() as ctx:
        cons = ctx.enter_context(tc.tile_pool(name="cons", bufs=1))
        xp = ctx.enter_context(tc.tile_pool(name="xp", bufs=3))
        rp = ctx.enter_context(tc.tile_pool(name="rp", bufs=4))
        pp = ctx.enter_context(tc.tile_pool(name="pp", bufs=2, space="PSUM"))
        w1 = cons.tile([128, KC, 80], F16)
        nc.sync.dma_start(out=w1[:], in_=w1r[:])
        for b in range(NBLK):
            xt = xp.tile([128, KC, 128], F16)
            nc.sync.dma_start(out=xt[:], in_=xT[:, :, b * 128:(b + 1) * 128])
            ps = pp.tile([128, 80], F32)
            for kc in range(KC):
                nc.tensor.matmul(out=ps[:], lhsT=xt[:, kc, :],
                                 rhs=w1[:, kc, :],
                                 start=(kc == 0), stop=(kc == KC - 1))
            row = rp.tile([128, ROW], F16, tag="row")
            nc.vector.memset(row[:], 0.0)
            nc.scalar.copy(row[:, 0:A1], ps[:, 0:A1])
            nc.sync.dma_start(out=t1[b * 128:(b + 1) * 128, :], in_=row[:])
            row2 = rp.tile([128, H1], F16, tag="row2")
            nc.scalar.copy(row2[:], ps[:, A1:A1 + H1])
            nc.sync.dma_start(out=ad1[b * 128:(b + 1) * 128, :], in_=row2[:])
    nc.compile()
    return nc


def build_edge_layer(sched, layer):
    """layer 1: GAT1 edge stage + z projection (outputs table2 + ad2).
    layer 2: GAT2 edge stage + log_softmax (outputs res [NPER, 40] f32)."""
    bacc, bass, tile, mybir = _bass_mods()
    from contextlib import ExitStack
    F16, F32, I16 = mybir.dt.float16, mybir.dt.float32, mybir.dt.int16
    AF = mybir.AluOpType

    NH = H1 if layer == 1 else 1         # heads
    DF = D1 if layer == 1 else D2        # feature cols in gather row
    UW = A1 if layer == 1 else W2COLS    # psum U width (72 / 42+1?)

    nc = bacc.Bacc("TRN2", target_bir_lowering=False)
    tfull = nc.dram_tensor("tfull", [NPAD, ROW], F16, kind="ExternalInput")
    adt = nc.dram_tensor("adt", [NPER, NH], F16, kind="ExternalInput")
    gidx_d = nc.dram_tensor("gidx", [128, sched.nslot // 16], I16,
                            kind="ExternalInput")
    doT_d = nc.dram_tensor("doT", [1, sched.ntiles * 128], F16,
                           kind="ExternalInput")
    if layer == 1:
        do_d = nc.dram_tensor("do16", [128, sched.ntiles], F16,
                              kind="ExternalInput")
        w2r_d = nc.dram_tensor("w2r", [64, W2COLS], F16, kind="ExternalInput")
        b1_d = nc.dram_tensor("b1v", [1, D1], F32, kind="ExternalInput")
        t2 = nc.dram_tensor("t2s", [NPER, ROW], F16, kind="ExternalOutput")
        ad2 = nc.dram_tensor("ad2s", [NPER, 1], F16, kind="ExternalOutput")
    else:
        do_d = nc.dram_tensor("do16", [128, sched.ntiles], F16,
                              kind="ExternalInput")
        b2_d = nc.dram_tensor("b2v", [1, D2], F32, kind="ExternalInput")
        res = nc.dram_tensor("res", [NPER, D2], F32, kind="ExternalOutput")

    ntmax = sched.ntmax
    with tile.TileContext(nc) as tc, ExitStack() as ctx:
        cons = ctx.enter_context(tc.tile_pool(name="cons", bufs=1))
        gp = ctx.enter_context(tc.tile_pool(name="gp", bufs=3))
        ip = ctx.enter_context(tc.tile_pool(name="ip", bufs=3))
        dop = ctx.enter_context(tc.tile_pool(name="dop", bufs=2))
        dtp = ctx.enter_context(tc.tile_pool(name="dtp", bufs=2))
        mtp = ctx.enter_context(tc.tile_pool(name="mtp", bufs=2))
        mkp = ctx.enter_context(tc.tile_pool(name="mkp", bufs=4))
        rhp = ctx.enter_context(tc.tile_pool(name="rhp", bufs=4))
        stp = ctx.enter_context(tc.tile_pool(name="stp", bufs=2))
        bp = ctx.enter_context(tc.tile_pool(name="bp", bufs=3))
        psA = ctx.enter_context(tc.tile_pool(name="psA", bufs=2, space="PSUM"))
        psU = ctx.enter_context(tc.tile_pool(name="psU", bufs=2, space="PSUM"))
        ps1 = ctx.enter_context(tc.tile_pool(name="ps1", bufs=2, space="PSUM"))

        # constants
        iota16 = cons.tile([128, TB, 128], I16)
        nc.gpsimd.iota(iota16[:], pattern=[[0, TB], [1, 128]], base=0,
                       channel_multiplier=0)
        iotaF = cons.tile([128, TB, 128], F16)
        nc.vector.tensor_copy(out=iotaF[:], in_=iota16[:])
        iotaP16 = cons.tile([128, 128], I16)
        nc.gpsimd.iota(iotaP16[:], pattern=[[0, 128]], base=0,
                       channel_multiplier=1)
        iotaP = cons.tile([128, 128], F16)
        nc.vector.tensor_copy(out=iotaP[:], in_=iotaP16[:])
        # all al_dst rows of this core: [128, NBLK, NH]
        adtAll = cons.tile([128, NBLK, NH], F16)
        nc.sync.dma_start(
            out=adtAll[:],
            in_=adt[:].rearrange("(b p) h -> p b h", p=128))
        if layer == 1:
            from concourse.masks import make_identity
            ident = cons.tile([128, 128], F16)
            make_identity(nc, ident[:])
            w2s = cons.tile([64, W2COLS], F16)
            nc.sync.dma_start(out=w2s[:], in_=w2r_d[:])
            bB = cons.tile([128, D1], F32)
            nc.sync.dma_start(out=bB[:], in_=b1_d[0:1, :].to_broadcast(
                [128, D1]))
        else:
            bB = cons.tile([128, D2], F32)
            nc.sync.dma_start(out=bB[:], in_=b2_d[0:1, :].to_broadcast(
                [128, D2]))
            omAll = cons.tile([128, NBLK, D2], F32)   # staged for final pass
            smAll = cons.tile([128, NBLK], F32)

        _maxsb = int(os.environ.get("GAT_MAX_SB", "0"))
        sbs_iter = sched.sbs[:_maxsb] if _maxsb else sched.sbs
        for sb in sbs_iter:
            gts = []
            for c in range(CHUNKS):
                ntc = sb["ntc"][c]
                if ntc == 0:
                    gts.append(None)
                    continue
                gi = ip.tile([128, ntc * 8], I16, tag=f"gi{c}")
                nc.sync.dma_start(
                    out=gi[:],
                    in_=gidx_d[:, sb["gbase"][c] * 8:
                               (sb["gbase"][c] + ntc) * 8])
                gt = gp.tile([128, ntc, ROW], F16, tag=f"g{c}")
                nc.gpsimd.dma_gather(
                    gt[:], tfull[c * CROWS:(c + 1) * CROWS, :], gi[:],
                    ntc * 128, ntc * 128, ROW, single_packet=False)
                gts.append(gt)
            do = dop.tile([128, sb["ntiles"]], F16)
            nc.sync.dma_start(
                out=do[:], in_=do_d[:, sb["tpbase"]:
                                    sb["tpbase"] + sb["ntiles"]])

            _stage = int(os.environ.get("GAT_STAGE", "5"))
            if _stage < 1:
                continue
            for b, tiles in sb["blocks"]:
                ntb = len(tiles)
                if ntb == 0:
                    continue
                if _stage < 2:
                    continue
                # --- al_dst per edge: transposed one-hot mask x local ad
                #     rows.  mkT[d, e] = (doT[e] == d); one DVE op per block,
                #     one K=128 matmul (N=NH) per tile. ---
                t0g = sb["tpbase"] + tiles[0][2]     # global first tile
                doTb = dtp.tile([128, ntb * 128], F16)
                nc.sync.dma_start(
                    out=doTb[:],
                    in_=doT_d[0:1, t0g * 128:(t0g + ntb) * 128].to_broadcast(
                        [128, ntb * 128]))
                mkT = mtp.tile([128, ntb * 128], F16)
                nc.vector.tensor_tensor(
                    out=mkT[:].rearrange("p (t e) -> p t e", e=128),
                    in0=doTb[:].rearrange("p (t e) -> p t e", e=128),
                    in1=iotaP[:].unsqueeze(1).to_broadcast(
                        [128, ntb, 128]),
                    op=AF.is_equal)
                adps = psA.tile([128, max(ntmax * NH, 8)], F32)
                for tl in range(ntb):
                    nc.tensor.matmul(
                        out=adps[:, tl * NH:(tl + 1) * NH],
                        lhsT=mkT[:, tl * 128:(tl + 1) * 128],
                        rhs=adtAll[:, b, :],
                        start=True, stop=True)
                if _stage < 3:
                    continue
                # --- staged e / ex ---
                asf = stp.tile([128, max(ntmax * NH, 8)], F32, tag="asf")
                a3 = asf[:].rearrange("p (t h) -> p t h", h=NH)
                tl0 = 0
                for c in range(CHUNKS):
                    ctiles = [x for x in tiles if x[0] == c]
                    if not ctiles:
                        continue
                    s0 = ctiles[0][1]
                    ncn = len(ctiles)
                    nc.scalar.copy(
                        a3[:, tl0:tl0 + ncn, :],
                        gts[c][:, s0:s0 + ncn, DF:DF + NH])
                    tl0 += ncn
                ef = stp.tile([128, max(ntmax * NH, 8)], F32, tag="ef")
                nc.vector.tensor_add(out=ef[:, 0:ntb * NH],
                                     in0=asf[:, 0:ntb * NH],
                                     in1=adps[:, 0:ntb * NH])
                nc.vector.scalar_tensor_tensor(
                    out=ef[:, 0:ntb * NH], in0=ef[:, 0:ntb * NH],
                    scalar=NEG_SLOPE, in1=ef[:, 0:ntb * NH],
                    op0=AF.mult, op1=AF.max)
                nc.vector.tensor_scalar(
                    out=ef[:, 0:ntb * NH], in0=ef[:, 0:ntb * NH],
                    scalar1=CLAMP, scalar2=None, op0=AF.min)
                ex = stp.tile([128, max(ntmax * NH, 8)], F16, tag="ex")
                nc.scalar.activation(ex[:, 0:ntb * NH], ef[:, 0:ntb * NH],
                                     mybir.ActivationFunctionType.Exp)

                if _stage < 4:
                    continue
                # --- masks + weighted aggregation (batch TB tiles per DVE
                #     op; one matmul per tile accumulating into U|S) ---
                ups = psU.tile([128, UW], F32)
                tl = 0
                for c in range(CHUNKS):
                    ctiles = [x for x in tiles if x[0] == c]
                    i = 0
                    while i < len(ctiles):
                        nb = min(TB, len(ctiles) - i)
                        c0, s0, tg0 = ctiles[i]
                        tcol = tg0
                        mk = mkp.tile([128, TB, 128], F16)
                        nc.vector.tensor_tensor(
                            out=mk[:, 0:nb, :],
                            in0=iotaF[:, 0:nb, :],
                            in1=do[:, tcol:tcol + nb].unsqueeze(
                                2).to_broadcast([128, nb, 128]),
                            op=AF.is_equal)
                        rhs = rhp.tile([128, TB, UW], F16)
                        if layer == 1:
                            g4 = gts[c][:, s0:s0 + nb, 0:D1].rearrange(
                                "p t (h c2) -> p t h c2", h=H1)
                            e4 = ex[:, tl * H1:(tl + nb) * H1].rearrange(
                                "p (t h) -> p t h", h=H1).unsqueeze(
                                    3).to_broadcast([128, nb, H1, C1])
                            nc.vector.tensor_tensor(
                                out=rhs[:, 0:nb, 0:D1].rearrange(
                                    "p t (h c2) -> p t h c2", h=H1),
                                in0=g4, in1=e4, op=AF.mult)
                            nc.scalar.copy(
                                out=rhs[:, 0:nb, D1:A1],
                                in_=ex[:, tl * H1:(tl + nb) * H1]
                                .rearrange("p (t h) -> p t h", h=NH))
                        else:
                            e4 = ex[:, tl:tl + nb].unsqueeze(
                                2).to_broadcast([128, nb, W2COLS])
                            nc.vector.tensor_tensor(
                                out=rhs[:, 0:nb, :],
                                in0=gts[c][:, s0:s0 + nb, 0:W2COLS],
                                in1=e4, op=AF.mult)
                        for j in range(nb):
                            nc.tensor.matmul(
                                out=ups[:], lhsT=mk[:, j, :],
                                rhs=rhs[:, j, :],
                                start=(tl + j == 0),
                                stop=(tl + j == ntb - 1))
                        tl += nb
                        i += nb

                if _stage < 5:
                    continue
                # --- block epilogue ---
                if layer == 1:
                    seps = bp.tile([128, H1], F32, tag="seps")
                    nc.vector.tensor_scalar(
                        out=seps[:], in0=ups[:, D1:A1], scalar1=1e-16,
                        scalar2=None, op0=AF.add)
                    sinv = bp.tile([128, H1], F32, tag="sinv")
                    nc.vector.reciprocal(sinv[:], seps[:])
                    o1 = bp.tile([128, D1], F32, tag="o1")
                    nc.vector.tensor_tensor(
                        out=o1[:].rearrange("p (h c2) -> p h c2", h=H1),
                        in0=ups[:, 0:D1].rearrange(
                            "p (h c2) -> p h c2", h=H1),
                        in1=sinv[:].unsqueeze(2).to_broadcast(
                            [128, H1, C1]),
                        op=AF.mult)
                    nc.vector.tensor_add(out=o1[:], in0=o1[:], in1=bB[:])
                    o1f = bp.tile([128, D1], F16, tag="o1f")
                    nc.scalar.activation(o1f[:], o1[:],
                                         mybir.ActivationFunctionType.Relu)
                    o1tp = ps1.tile([64, 128], F16, tag="o1tp")
                    nc.tensor.transpose(out=o1tp[:], in_=o1f[:],
                                        identity=ident[:])
                    o1t = bp.tile([64, 128], F16, tag="o1t")
                    nc.vector.tensor_copy(out=o1t[:], in_=o1tp[:])
                    zps = ps1.tile([128, W2COLS], F32, tag="zps")
                    nc.tensor.matmul(out=zps[:], lhsT=o1t[:], rhs=w2s[:],
                                     start=True, stop=True)
                    row = bp.tile([128, ROW], F16, tag="row")
                    nc.vector.memset(row[:], 0.0)
                    nc.scalar.copy(row[:, 0:D2 + 1], zps[:, 0:D2 + 1])
                    nc.vector.memset(row[:, D2 + 1:D2 + 2], 1.0)
                    nc.sync.dma_start(out=t2[b * 128:(b + 1) * 128, :],
                                      in_=row[:])
                    row2 = bp.tile([128, 1], F16, tag="row2")
                    nc.scalar.copy(row2[:], zps[:, D2 + 1:D2 + 2])
                    nc.sync.dma_start(out=ad2[b * 128:(b + 1) * 128, :],
                                      in_=row2[:])
                else:
                    seps = bp.tile([128, 1], F32, tag="seps")
                    nc.vector.tensor_scalar(
                        out=seps[:], in0=ups[:, D2 + 1:D2 + 2], scalar1=1e-16,
                        scalar2=None, op0=AF.add)
                    sinv = bp.tile([128, 1], F32, tag="sinv")
                    nc.vector.reciprocal(sinv[:], seps[:])
                    o = bp.tile([128, D2], F32, tag="o")
                    nc.vector.tensor_tensor(
                        out=o[:], in0=ups[:, 0:D2],
                        in1=sinv[:].to_broadcast([128, D2]), op=AF.mult)
                    nc.vector.tensor_add(out=o[:], in0=o[:], in1=bB[:])
                    mx = bp.tile([128, 1], F32, tag="mx")
                    nc.vector.reduce_max(out=mx[:], in_=o[:],
                                         axis=mybir.AxisListType.X)
                    nc.vector.tensor_tensor(
                        out=omAll[:, b, :], in0=o[:],
                        in1=mx[:, 0:1].to_broadcast([128, D2]),
                        op=AF.subtract)
                    scr = bp.tile([128, D2], F32, tag="scr")
                    nc.scalar.activation(scr[:], omAll[:, b, :],
                                         mybir.ActivationFunctionType.Exp,
                                         accum_out=smAll[:, b:b + 1])
        if layer == 2 and not _maxsb:
            # final pass: one Ln over all blocks, subtract, write out
            lnt = cons.tile([128, NBLK], F32)
            nc.scalar.activation(lnt[:], smAll[:],
                                 mybir.ActivationFunctionType.Ln)
            for b in range(NBLK):
                ot = bp.tile([128, D2], F32, tag="ot")
                nc.vector.tensor_tensor(
                    out=ot[:], in0=omAll[:, b, :],
                    in1=lnt[:, b:b + 1].to_broadcast([128, D2]),
                    op=AF.subtract)
                nc.sync.dma_start(out=res[b * 128:(b + 1) * 128, :],
                                  in_=ot[:])
    nc.compile()
    return nc


# ------------------------------------------------------------------- runner

def _run(nc, in_maps, trace):
    from concourse.bass_utils import run_bass_kernel_spmd
    LAST_RUNS.append((nc, in_maps))
    try:
        r = run_bass_kernel_spmd(nc, in_maps, core_ids=list(range(NC_CORES)),
                                 trace=trace)
    except ModuleNotFoundError:
        r = run_bass_kernel_spmd(nc, in_maps, core_ids=list(range(NC_CORES)),
                                 trace=False)
    if r.exec_time_ns is not None:
        LAST_EXEC_TIMES.append(r.exec_time_ns)
    if r.instructions_and_trace is not None:
        LAST_TRACES.append(r.instructions_and_trace[1])
    return r.results


def kernel(x, edge_index, W1, a_src1, a_dst1, b1, W2, a_src2, a_dst2, b2):
    trace = bool(int(os.environ.get("GAT_TRACE", "0")))
    LAST_EXEC_TIMES.clear()
    LAST_RUNS.clear()
    LAST_TRACES.clear()

    sched, streams, w1r, xT_cores, w2r = _prepare(
        x, edge_index, W1, a_src1, a_dst1, W2, a_src2, a_dst2)

    # ---- P1
    nc1 = build_p1()
    in1 = [dict(xT=xT_cores[k], w1r=w1r) for k in range(NC_CORES)]
    r1 = _run(nc1, in1, trace)
    t1_full = np.concatenate([r1[k]["t1s"] for k in range(NC_CORES)], axis=0)
    ad1s = [r1[k]["ad1s"] for k in range(NC_CORES)]

    # ---- P2
    b1v = np.asarray(b1, np.float32).reshape(1, D1)
    nc2 = build_edge_layer(sched, 1)
    in2 = [dict(tfull=t1_full, adt=ad1s[k], gidx=streams[k]["gidx"],
                doT=streams[k]["doT16"], do16=streams[k]["do16"],
                w2r=w2r, b1v=b1v)
           for k in range(NC_CORES)]
    r2 = _run(nc2, in2, trace)
    t2_full = np.concatenate([r2[k]["t2s"] for k in range(NC_CORES)], axis=0)
    ad2s = [r2[k]["ad2s"] for k in range(NC_CORES)]

    # ---- P3
    b2v = np.asarray(b2, np.float32).reshape(1, D2)
    nc3 = build_edge_layer(sched, 2)
    in3 = [dict(tfull=t2_full, adt=ad2s[k], gidx=streams[k]["gidx"],
                doT=streams[k]["doT16"], do16=streams[k]["do16"], b2v=b2v)
           for k in range(NC_CORES)]
    r3 = _run(nc3, in3, trace)
    out = np.concatenate([r3[k]["res"] for k in range(NC_CORES)], axis=0)
    return out[:N].astype(np.float32)


# revision 15
# speedup vs baseline: 1.6191x; 1.0485x over previous
# GATNet (2-layer GAT, 8 heads x 8 then 1 head x 40) on 8 trn2 NeuronCores.
#
# Strategy (dst-sharded graph parallel, 3 SPMD launches):
#   P1: per-core node projection of its 1/8 node shard:
#       table1 row  = [h1(64) | al_src1(8) | pad]  (fp16, 128 elems = 256B)
#       ad1 row     = [al_dst1(8)]                 (fp16, 16B)
#       (host concatenates the 8 table1 shards into the full gather table)
#   P2: per-core edge stage for GAT layer 1 over the core's dst-owned edges
#       (dst-sorted, grouped in 128-dst blocks, 4 src-chunk substreams for
#       int16 gather indices). Per 128-edge tile:
#         - dma_gather of table1 rows by src (256B/edge) [SWDGE, GpSimd]
#         - al_dst per edge via transposed one-hot mask: mkT[d,e] =
#           (doT[e] == p_iota[d]) built in one DVE op per block, then a
#           K=128 matmul vs the block's local ad rows (adtAll[:, b, :])
#         - e = leaky_relu(al_src + al_dst); ex = exp(min(e, 11)) (staged)
#         - mask[e,d] = (dstoff[e] == iota[d]) built on DVE, 4 tiles/op
#         - PSUM accumulate [U|S] = mask.T @ [ex*h | ex] per dst block
#       Block epilogue: out1 = relu(U/S + b1); z = out1 @ [W2|W2 a_src2|W2 a_dst2]
#       -> table2 row = [z(40) | as2 | 1.0 | pad] fp16, ad2 [NPER,1] (outputs).
#   P3: same edge stage for layer 2 (H=1): rhs = gathered row * ex (DVE),
#       U2|S2 = mask.T @ rhs, epilogue log_softmax (batched Ln) -> out.
#
# The edge schedule (tiles per (block, chunk)) is shared by all 8 cores
# (max over cores), so one NEFF per launch runs SPMD with per-core inputs.
#
# Perf notes (measured on trn2 via NTFF traces):
#   - dma_gather (SWDGE) costs ~8.6 ns/row descriptor, serialized on the
#     GpSimd engine (only one Q7 cpu pair generates descriptors; DMA
#     engines themselves are <10% busy).  At ~451K gathered rows per core
#     per edge stage this is the ~4.2 ms/stage floor of this design.
#   - gpsimd.ap_gather (SBUF-source, through-DSP) measures 0.78 ns/idx
#     moving 128ch x 4B per idx — 11x faster than SWDGE per row.  A
#     chunk-outer redesign holding transposed 100KB/partition table
#     slices in SBUF and gathering feature-major via ap_gather (plus two
#     fp16 PE transposes per 128-edge tile) would cut each edge stage to
#     ~2.5 ms.  Not implemented here (ran out of session budget).
#   - DVE tensor ops with a stride-0 broadcast on the MAJOR free dim run
#     ~10x slow; broadcast on middle/inner dims is fast.  Avoid AP-scalar
#     tensor_scalar with two scalar operands (2.3 us per [128,128] op).

import math
import os
import numpy as np

N = 100000
E = 3200000
F_IN = 512
H1, C1 = 8, 8
D1 = H1 * C1            # 64
D2 = 40                 # 1 head x 40
NEG_SLOPE = 0.2
CLAMP = 11.0            # exp clamp so fp16 ex stays finite (e>11 has P~1e-8)

NC_CORES = 8
NPER = 12544            # 98 * 128 owned dsts per core (>= ceil(100000/8))
NBLK = NPER // 128      # 98
NPAD = NPER * NC_CORES  # 100352
CHUNKS = 4
CROWS = NPAD // CHUNKS  # 25088 (< 32767 so int16 gather idxs work)
ROW = 128               # fp16 elems per gather-table row (256B)
SBB = 4                 # dst blocks per superblock (gather granularity)
A1 = D1 + H1            # 72: [h | al_src]
W2COLS = D2 + 2         # 42: [W2 | W2 a_src2 | W2 a_dst2]
TB = 4                  # tiles batched per DVE mask/rhs op

_NC_CACHE = {}
LAST_EXEC_TIMES = []    # test.py reads this (ns per launch, if traced)
LAST_RUNS = []          # (nc, in_maps) per launch, for test.py timing
LAST_TRACES = []        # perfetto trace paths per launch, if traced


# ----------------------------------------------------------------- host prep

def _wrap16(a):
    """Edge-stream array (len % 16 == 0) -> [128, len//16] int16 wrapped
    layout: position i lives at [i % 16, i // 16], replicated to 8x16 rows."""
    a = np.asarray(a, dtype=np.int16)
    w = a.reshape(-1, 16).T  # [16, len//16]
    return np.tile(w, (8, 1)).copy()


def _schedule(counts):
    """counts: [NC, NBLK, CHUNKS] real edge counts. Returns T [NBLK][CHUNKS]
    tiles per (block, chunk), shared across cores."""
    mx = counts.max(axis=0)  # [NBLK, CHUNKS]
    T = np.ceil(mx / 128).astype(np.int64)
    return T


class Sched:
    """Static schedule metadata shared by all cores (drives program build and
    host stream packing identically)."""

    def __init__(self, T, sbb=SBB):
        self.T = T                       # [NBLK][CHUNKS]
        nblk = T.shape[0]
        self.sbs = []                    # list of dicts
        gbase = 0                        # gather stream base, slots/128 units
        tpbase = 0                       # processing-order tile base
        for s0 in range(0, nblk, sbb):
            blocks = list(range(s0, min(s0 + sbb, nblk)))
            ntc = [int(T[blocks, c].sum()) for c in range(CHUNKS)]
            gb = []
            for c in range(CHUNKS):
                gb.append(gbase)
                gbase += ntc[c]
            ntiles = sum(ntc)
            binfo = []
            tg = 0
            for b in blocks:
                tiles = []
                for c in range(CHUNKS):
                    # slot index of (b,c,t) inside gt[c] for this sb
                    off = int(T[[bb for bb in blocks if bb < b], c].sum())
                    for t in range(int(T[b, c])):
                        tiles.append((c, off + t, tg))
                        tg += 1
                binfo.append((b, tiles))
            self.sbs.append(dict(
                blocks=binfo, ntc=ntc, gbase=gb, ntiles=ntiles,
                tpbase=tpbase))
            tpbase += ntiles
        self.ntiles = tpbase             # total (processing-order) tiles
        self.nslot = int(T.sum()) * 128  # total gather slots
        self.ntmax = max(
            sum(int(T[b, c]) for c in range(CHUNKS))
            for b in range(nblk))        # max tiles per block
        self.ntcmax = max(max(sb["ntc"]) for sb in self.sbs)


def _prep_core(k, src, dst, T, sched):
    """Build one core's edge streams. src/dst: this core's edges (dst local)."""
    blk = dst // 128
    doff = dst % 128
    ch = src // CROWS
    order = np.lexsort((doff, ch, blk))
    src, doff, ch, blk = src[order], doff[order], ch[order], blk[order]
    # group boundaries per (blk, ch)
    nblk = T.shape[0]
    key = blk * CHUNKS + ch
    starts = np.searchsorted(key, np.arange(nblk * CHUNKS))
    ends = np.searchsorted(key, np.arange(nblk * CHUNKS) + 1)

    gidx = np.zeros(sched.nslot, np.int16)
    do16 = np.full((128, sched.ntiles), -1.0, np.float16)
    do32 = np.full((128, sched.ntiles), -1.0, np.float32)
    doT16 = np.full((1, sched.ntiles * 128), -1.0, np.float16)

    gpos = 0
    for sb in sched.sbs:
        # gather-stream order: (c, b, t); processing order: (b, c, t)
        for c in range(CHUNKS):
            for b, tiles in sb["blocks"]:
                s, e = starts[b * CHUNKS + c], ends[b * CHUNKS + c]
                nsl = int(T[b, c]) * 128
                if nsl == 0:
                    continue
                n = e - s
                gidx[gpos:gpos + n] = (src[s:e] % CROWS).astype(np.int16)
                ctiles = [x for x in tiles if x[0] == c]  # t asc
                for t, (_, slot, tg) in enumerate(ctiles):
                    lo, hi = t * 128, min((t + 1) * 128, n)
                    if hi <= lo:
                        continue
                    m = hi - lo
                    seg = doff[s + lo:s + hi]
                    tcol = sb["tpbase"] + tg
                    do16[:m, tcol] = seg.astype(np.float16)
                    do32[:m, tcol] = seg.astype(np.float32)
                    doT16[0, tcol * 128:tcol * 128 + m] = seg.astype(
                        np.float16)
                gpos += nsl
    assert gpos == sched.nslot
    return dict(gidx=_wrap16(gidx), do16=do16, do32=do32, doT16=doT16)


def _prepare(x, edge_index, W1, a_src1, a_dst1, W2, a_src2, a_dst2):
    src = np.concatenate([np.asarray(edge_index[0]),
                          np.arange(N, dtype=np.int64)]).astype(np.int64)
    dst = np.concatenate([np.asarray(edge_index[1]),
                          np.arange(N, dtype=np.int64)]).astype(np.int64)
    owner = dst // NPER
    per_core = []
    counts = np.zeros((NC_CORES, NBLK, CHUNKS), np.int64)
    for k in range(NC_CORES):
        m = owner == k
        s_k = src[m]
        d_k = dst[m] - k * NPER
        per_core.append((s_k, d_k))
        b = d_k // 128
        c = s_k // CROWS
        np.add.at(counts[k], (b, c), 1)
    T = _schedule(counts)
    sched = Sched(T)

    streams = [_prep_core(k, s_k, d_k, T, sched)
               for k, (s_k, d_k) in enumerate(per_core)]

    # P1 inputs
    W1f = np.asarray(W1, np.float32)                       # [512, 64]
    W1as = np.stack([W1f[:, h * C1:(h + 1) * C1] @ np.asarray(a_src1)[h]
                     for h in range(H1)], axis=1)          # [512, 8]
    W1ad = np.stack([W1f[:, h * C1:(h + 1) * C1] @ np.asarray(a_dst1)[h]
                     for h in range(H1)], axis=1)
    W1R = np.concatenate([W1f, W1as, W1ad], axis=1)        # [512, 80]
    w1r = W1R.reshape(F_IN // 128, 128, 80).transpose(1, 0, 2).astype(
        np.float16).copy()                                 # [128, KC, 80]

    xf = np.zeros((NPAD, F_IN), np.float32)
    xf[:N] = np.asarray(x, np.float32)
    xT_cores = []
    for k in range(NC_CORES):
        xs = xf[k * NPER:(k + 1) * NPER]                   # [NPER, 512]
        xt = xs.T.reshape(F_IN // 128, 128, NPER).transpose(1, 0, 2)
        xT_cores.append(np.ascontiguousarray(xt, dtype=np.float16))

    # P2 consts
    W2f = np.asarray(W2, np.float32)                       # [64, 40]
    w2r = np.concatenate(
        [W2f, W2f @ np.asarray(a_src2)[0][:, None],
         W2f @ np.asarray(a_dst2)[0][:, None]], axis=1).astype(np.float16)

    return sched, streams, w1r, xT_cores, w2r


# ------------------------------------------------------------- bass builders

def _bass_mods():
    import concourse.bacc as bacc
    import concourse.bass as bass
    import concourse.tile as tile
    import concourse.mybir as mybir
    return bacc, bass, tile, mybir


def build_p1():
    bacc, bass, tile, mybir = _bass_mods()
    from contextlib import ExitStack
    F16, F32 = mybir.dt.float16, mybir.dt.float32
    KC = F_IN // 128

    nc = bacc.Bacc("TRN2", target_bir_lowering=False)
    xT = nc.dram_tensor("xT", [128, KC, NPER], F16, kind="ExternalInput")
    w1r = nc.dram_tensor("w1r", [128, KC, 80], F16, kind="ExternalInput")
    t1 = nc.dram_tensor("t1s", [NPER, ROW], F16, kind="ExternalOutput")
    ad1 = nc.dram_tensor("ad1s", [NPER, H1], F16, kind="ExternalOutput")
    with tile.TileContext(nc) as tc, ExitStack() as ctx:
        cons = ctx.enter_context(tc.tile_pool(name="cons", bufs=1))
        xp = ctx.enter_context(tc.tile_pool(name="xp", bufs=3))
        rp = ctx.enter_context(tc.tile_pool(name="rp", bufs=4))
        pp = ctx.enter_context(tc.tile_pool(name="pp", bufs=2, space="PSUM"))
        w1 = cons.tile([128, KC, 80], F16)
        nc.sync.dma_start(out=w1[:], in_=w1r[:])
        for b in range(NBLK):
            xt = xp.tile([128, KC, 128], F16)
            nc.sync.dma_start(out=xt[:], in_=xT[:, :, b * 128:(b + 1) * 128])
            ps = pp.tile([128, 80], F32)
            for kc in range(KC):
                nc.tensor.matmul(out=ps[:], lhsT=xt[:, kc, :],
                                 rhs=w1[:, kc, :],
                                 start=(kc == 0), stop=(kc == KC - 1))
            row = rp.tile([128, ROW], F16, tag="row")
            nc.vector.memset(row[:], 0.0)
            nc.scalar.copy(row[:, 0:A1], ps[:, 0:A1])
            nc.sync.dma_start(out=t1[b * 128:(b + 1) * 128, :], in_=row[:])
            row2 = rp.tile([128, H1], F16, tag="row2")
            nc.scalar.copy(row2[:], ps[:, A1:A1 + H1])
            nc.sync.dma_start(out=ad1[b * 128:(b + 1) * 128, :], in_=row2[:])
    nc.compile()
    return nc


def build_edge_layer(sched, layer):
    """layer 1: GAT1 edge stage + z projection (outputs table2 + ad2).
    layer 2: GAT2 edge stage + log_softmax (outputs res [NPER, 40] f32)."""
    bacc, bass, tile, mybir = _bass_mods()
    from contextlib import ExitStack
    F16, F32, I16 = mybir.dt.float16, mybir.dt.float32, mybir.dt.int16
    AF = mybir.AluOpType

    NH = H1 if layer == 1 else 1         # heads
    DF = D1 if layer == 1 else D2        # feature cols in gather row
    UW = A1 if layer == 1 else W2COLS    # psum U width (72 / 42+1?)

    nc = bacc.Bacc("TRN2", target_bir_lowering=False)
    tfull = nc.dram_tensor("tfull", [NPAD, ROW], F16, kind="ExternalInput")
    adt = nc.dram_tensor("adt", [NPER, NH], F16, kind="ExternalInput")
    gidx_d = nc.dram_tensor("gidx", [128, sched.nslot // 16], I16,
                            kind="ExternalInput")
    doT_d = nc.dram_tensor("doT", [1, sched.ntiles * 128], F16,
                           kind="ExternalInput")
    if layer == 1:
        do_d = nc.dram_tensor("do16", [128, sched.ntiles], F16,
                              kind="ExternalInput")
        w2r_d = nc.dram_tensor("w2r", [64, W2COLS], F16, kind="ExternalInput")
        b1_d = nc.dram_tensor("b1v", [1, D1], F32, kind="ExternalInput")
        t2 = nc.dram_tensor("t2s", [NPER, ROW], F16, kind="ExternalOutput")
        ad2 = nc.dram_tensor("ad2s", [NPER, 1], F16, kind="ExternalOutput")
    else:
        do_d = nc.dram_tensor("do16", [128, sched.ntiles], F16,
                              kind="ExternalInput")
        b2_d = nc.dram_tensor("b2v", [1, D2], F32, kind="ExternalInput")
        res = nc.dram_tensor("res", [NPER, D2], F32, kind="ExternalOutput")

    ntmax = sched.ntmax
    with tile.TileContext(nc) as tc, ExitStack() as ctx:
        cons = ctx.enter_context(tc.tile_pool(name="cons", bufs=1))
        gp = ctx.enter_context(tc.tile_pool(name="gp", bufs=3))
        ip = ctx.enter_context(tc.tile_pool(name="ip", bufs=3))
        dop = ctx.enter_context(tc.tile_pool(name="dop", bufs=2))
        dtp = ctx.enter_context(tc.tile_pool(name="dtp", bufs=2))
        mtp = ctx.enter_context(tc.tile_pool(name="mtp", bufs=2))
        mkp = ctx.enter_context(tc.tile_pool(name="mkp", bufs=4))
        rhp = ctx.enter_context(tc.tile_pool(name="rhp", bufs=4))
        stp = ctx.enter_context(tc.tile_pool(name="stp", bufs=2))
        bp = ctx.enter_context(tc.tile_pool(name="bp", bufs=3))
        psA = ctx.enter_context(tc.tile_pool(name="psA", bufs=2, space="PSUM"))
        psU = ctx.enter_context(tc.tile_pool(name="psU", bufs=2, space="PSUM"))
        ps1 = ctx.enter_context(tc.tile_pool(name="ps1", bufs=2, space="PSUM"))

        # constants
        iota16 = cons.tile([128, TB, 128], I16)
        nc.gpsimd.iota(iota16[:], pattern=[[0, TB], [1, 128]], base=0,
                       channel_multiplier=0)
        iotaF = cons.tile([128, TB, 128], F16)
        nc.vector.tensor_copy(out=iotaF[:], in_=iota16[:])
        iotaP16 = cons.tile([128, 128], I16)
        nc.gpsimd.iota(iotaP16[:], pattern=[[0, 128]], base=0,
                       channel_multiplier=1)
        iotaP = cons.tile([128, 128], F16)
        nc.vector.tensor_copy(out=iotaP[:], in_=iotaP16[:])
        # all al_dst rows of this core: [128, NBLK, NH]
        adtAll = cons.tile([128, NBLK, NH], F16)
        nc.sync.dma_start(
            out=adtAll[:],
            in_=adt[:].rearrange("(b p) h -> p b h", p=128))
        if layer == 1:
            from concourse.masks import make_identity
            ident = cons.tile([128, 128], F16)
            make_identity(nc, ident[:])
            w2s = cons.tile([64, W2COLS], F16)
            nc.sync.dma_start(out=w2s[:], in_=w2r_d[:])
            bB = cons.tile([128, D1], F32)
            nc.sync.dma_start(out=bB[:], in_=b1_d[0:1, :].to_broadcast(
                [128, D1]))
        else:
            bB = cons.tile([128, D2], F32)
            nc.sync.dma_start(out=bB[:], in_=b2_d[0:1, :].to_broadcast(
                [128, D2]))
            omAll = cons.tile([128, NBLK, D2], F32)   # staged for final pass
            smAll = cons.tile([128, NBLK], F32)

        _maxsb = int(os.environ.get("GAT_MAX_SB", "0"))
        sbs_iter = sched.sbs[:_maxsb] if _maxsb else sched.sbs
        for sb in sbs_iter:
            gts = []
            for c in range(CHUNKS):
                ntc = sb["ntc"][c]
                if ntc == 0:
                    gts.append(None)
                    continue
                gi = ip.tile([128, ntc * 8], I16, tag=f"gi{c}")
                nc.sync.dma_start(
                    out=gi[:],
                    in_=gidx_d[:, sb["gbase"][c] * 8:
                               (sb["gbase"][c] + ntc) * 8])
                gt = gp.tile([128, ntc, ROW], F16, tag=f"g{c}")
                nc.gpsimd.dma_gather(
                    gt[:], tfull[c * CROWS:(c + 1) * CROWS, :], gi[:],
                    ntc * 128, ntc * 128, ROW, single_packet=False)
                gts.append(gt)
            do = dop.tile([128, sb["ntiles"]], F16)
            nc.sync.dma_start(
                out=do[:], in_=do_d[:, sb["tpbase"]:
                                    sb["tpbase"] + sb["ntiles"]])

            _stage = int(os.environ.get("GAT_STAGE", "5"))
            if _stage < 1:
                continue
            for b, tiles in sb["blocks"]:
                ntb = len(tiles)
                if ntb == 0:
                    continue
                if _stage < 2:
                    continue
                # --- al_dst per edge: transposed one-hot mask x local ad
                #     rows.  mkT[d, e] = (doT[e] == d); one DVE op per block,
                #     one K=128 matmul (N=NH) per tile. ---
                t0g = sb["tpbase"] + tiles[0][2]     # global first tile
                doTb = dtp.tile([128, ntb * 128], F16)
                nc.sync.dma_start(
                    out=doTb[:],
                    in_=doT_d[0:1, t0g * 128:(t0g + ntb) * 128].to_broadcast(
                        [128, ntb * 128]))
                mkT = mtp.tile([128, ntb * 128], F16)
                nc.vector.tensor_tensor(
                    out=mkT[:].rearrange("p (t e) -> p t e", e=128),
                    in0=doTb[:].rearrange("p (t e) -> p t e", e=128),
                    in1=iotaP[:].unsqueeze(1).to_broadcast(
                        [128, ntb, 128]),
                    op=AF.is_equal)
                adps = psA.tile([128, max(ntmax * NH, 8)], F32)
                for tl in range(ntb):
                    nc.tensor.matmul(
                        out=adps[:, tl * NH:(tl + 1) * NH],
                        lhsT=mkT[:, tl * 128:(tl + 1) * 128],
                        rhs=adtAll[:, b, :],
                        start=True, stop=True)
                if _stage < 3:
                    continue
                # --- staged e / ex ---
                asf = stp.tile([128, max(ntmax * NH, 8)], F32, tag="asf")
                a3 = asf[:].rearrange("p (t h) -> p t h", h=NH)
                tl0 = 0
                for c in range(CHUNKS):
                    ctiles = [x for x in tiles if x[0] == c]
                    if not ctiles:
                        continue
                    s0 = ctiles[0][1]
                    ncn = len(ctiles)
                    nc.scalar.copy(
                        a3[:, tl0:tl0 + ncn, :],
                        gts[c][:, s0:s0 + ncn, DF:DF + NH])
                    tl0 += ncn
                ef = stp.tile([128, max(ntmax * NH, 8)], F32, tag="ef")
                nc.vector.tensor_add(out=ef[:, 0:ntb * NH],
                                     in0=asf[:, 0:ntb * NH],
                                     in1=adps[:, 0:ntb * NH])
                nc.vector.scalar_tensor_tensor(
                    out=ef[:, 0:ntb * NH], in0=ef[:, 0:ntb * NH],
                    scalar=NEG_SLOPE, in1=ef[:, 0:ntb * NH],
                    op0=AF.mult, op1=AF.max)
                nc.vector.tensor_scalar(
                    out=ef[:, 0:ntb * NH], in0=ef[:, 0:ntb * NH],
                    scalar1=CLAMP, scalar2=None, op0=AF.min)
                ex = stp.tile([128, max(ntmax * NH, 8)], F16, tag="ex")
                nc.scalar.activation(ex[:, 0:ntb * NH], ef[:, 0:ntb * NH],
                                     mybir.ActivationFunctionType.Exp)

                if _stage < 4:
                    continue
                # --- masks + weighted aggregation (batch TB tiles per DVE
                #     op; one matmul per tile accumulating into U|S) ---
                ups = psU.tile([128, UW], F32)
                tl = 0
                for c in range(CHUNKS):
                    ctiles = [x for x in tiles if x[0] == c]
                    i = 0
                    while i < len(ctiles):
                        nb = min(TB, len(ctiles) - i)
                        c0, s0, tg0 = ctiles[i]
                        tcol = tg0
                        mk = mkp.tile([128, TB, 128], F16)
                        nc.vector.tensor_tensor(
                            out=mk[:, 0:nb, :],
                            in0=iotaF[:, 0:nb, :],
                            in1=do[:, tcol:tcol + nb].unsqueeze(
                                2).to_broadcast([128, nb, 128]),
                            op=AF.is_equal)
                        rhs = rhp.tile([128, TB, UW], F16)
                        if layer == 1:
                            g4 = gts[c][:, s0:s0 + nb, 0:D1].rearrange(
                                "p t (h c2) -> p t h c2", h=H1)
                            e4 = ex[:, tl * H1:(tl + nb) * H1].rearrange(
                                "p (t h) -> p t h", h=H1).unsqueeze(
                                    3).to_broadcast([128, nb, H1, C1])
                            nc.vector.tensor_tensor(
                                out=rhs[:, 0:nb, 0:D1].rearrange(
                                    "p t (h c2) -> p t h c2", h=H1),
                                in0=g4, in1=e4, op=AF.mult)
                            nc.scalar.copy(
                                out=rhs[:, 0:nb, D1:A1],
                                in_=ex[:, tl * H1:(tl + nb) * H1]
                                .rearrange("p (t h) -> p t h", h=NH))
                        else:
                            e4 = ex[:, tl:tl + nb].unsqueeze(
                                2).to_broadcast([128, nb, W2COLS])
                            nc.vector.tensor_tensor(
                                out=rhs[:, 0:nb, :],
                                in0=gts[c][:, s0:s0 + nb, 0:W2COLS],
                                in1=e4, op=AF.mult)
                        for j in range(nb):
                            nc.tensor.matmul(
                                out=ups[:], lhsT=mk[:, j, :],
                                rhs=rhs[:, j, :],
                                start=(tl + j == 0),
                                stop=(tl + j == ntb - 1))
                        tl += nb
                        i += nb

                if _stage < 5:
                    continue
                # --- block epilogue ---
                if layer == 1:
                    seps = bp.tile([128, H1], F32, tag="seps")
                    nc.vector.tensor_scalar(
                        out=seps[:], in0=ups[:, D1:A1], scalar1=1e-16,
                        scalar2=None, op0=AF.add)
                    sinv = bp.tile([128, H1], F32, tag="sinv")
                    nc.vector.reciprocal(sinv[:], seps[:])
                    o1 = bp.tile([128, D1], F32, tag="o1")
                    nc.vector.tensor_tensor(
                        out=o1[:].rearrange("p (h c2) -> p h c2", h=H1),
                        in0=ups[:, 0:D1].rearrange(
                            "p (h c2) -> p h c2", h=H1),
                        in1=sinv[:].unsqueeze(2).to_broadcast(
                            [128, H1, C1]),
                        op=AF.mult)
                    nc.vector.tensor_add(out=o1[:], in0=o1[:], in1=bB[:])
                    o1f = bp.tile([128, D1], F16, tag="o1f")
                    nc.scalar.activation(o1f[:], o1[:],
                                         mybir.ActivationFunctionType.Relu)
                    o1tp = ps1.tile([64, 128], F16, tag="o1tp")
                    nc.tensor.transpose(out=o1tp[:], in_=o1f[:],
                                        identity=ident[:])
                    o1t = bp.tile([64, 128], F16, tag="o1t")
                    nc.vector.tensor_copy(out=o1t[:], in_=o1tp[:])
                    zps = ps1.tile([128, W2COLS], F32, tag="zps")
                    nc.tensor.matmul(out=zps[:], lhsT=o1t[:], rhs=w2s[:],
                                     start=True, stop=True)
                    row = bp.tile([128, ROW], F16, tag="row")
                    nc.vector.memset(row[:], 0.0)
                    nc.scalar.copy(row[:, 0:D2 + 1], zps[:, 0:D2 + 1])
                    nc.vector.memset(row[:, D2 + 1:D2 + 2], 1.0)
                    nc.sync.dma_start(out=t2[b * 128:(b + 1) * 128, :],
                                      in_=row[:])
                    row2 = bp.tile([128, 1], F16, tag="row2")
                    nc.scalar.copy(row2[:], zps[:, D2 + 1:D2 + 2])
                    nc.sync.dma_start(out=ad2[b * 128:(b + 1) * 128, :],
                                      in_=row2[:])
                else:
                    seps = bp.tile([128, 1], F32, tag="seps")
                    nc.vector.tensor_scalar(
                        out=seps[:], in0=ups[:, D2 + 1:D2 + 2], scalar1=1e-16,
                        scalar2=None, op0=AF.add)
                    sinv = bp.tile([128, 1], F32, tag="sinv")
                    nc.vector.reciprocal(sinv[:], seps[:])
                    o = bp.tile([128, D2], F32, tag="o")
                    nc.vector.tensor_tensor(
                        out=o[:], in0=ups[:, 0:D2],
                        in1=sinv[:].to_broadcast([128, D2]), op=AF.mult)
                    nc.vector.tensor_add(out=o[:], in0=o[:], in1=bB[:])
                    mx = bp.tile([128, 1], F32, tag="mx")
                    nc.vector.reduce_max(out=mx[:], in_=o[:],
                                         axis=mybir.AxisListType.X)
                    nc.vector.tensor_tensor(
                        out=omAll[:, b, :], in0=o[:],
                        in1=mx[:, 0:1].to_broadcast([128, D2]),
                        op=AF.subtract)
                    scr = bp.tile([128, D2], F32, tag="scr")
                    nc.scalar.activation(scr[:], omAll[:, b, :],
                                         mybir.ActivationFunctionType.Exp,
                                         accum_out=smAll[:, b:b + 1])
        if layer == 2 and not _maxsb:
            # final pass: one Ln over all blocks, subtract, write out
            lnt = cons.tile([128, NBLK], F32)
            nc.scalar.activation(lnt[:], smAll[:],
                                 mybir.ActivationFunctionType.Ln)
            for b in range(NBLK):
                ot = bp.tile([128, D2], F32, tag="ot")
                nc.vector.tensor_tensor(
                    out=ot[:], in0=omAll[:, b, :],
                    in1=lnt[:, b:b + 1].to_broadcast([128, D2]),
                    op=AF.subtract)
                nc.sync.dma_start(out=res[b * 128:(b + 1) * 128, :],
                                  in_=ot[:])
    nc.compile()
    return nc


# ------------------------------------------------------------------- runner

def _run(nc, in_maps, trace):
    from concourse.bass_utils import run_bass_kernel_spmd
    LAST_RUNS.append((nc, in_maps))
    try:
        r = run_bass_kernel_spmd(nc, in_maps, core_ids=list(range(NC_CORES)),
                                 trace=trace)
    except ModuleNotFoundError:
        r = run_bass_kernel_spmd(nc, in_maps, core_ids=list(range(NC_CORES)),
                                 trace=False)
    if r.exec_time_ns is not None:
        LAST_EXEC_TIMES.append(r.exec_time_ns)
    if r.instructions_and_trace is not None:
        LAST_TRACES.append(r.instructions_and_trace[1])
    return r.results


def kernel(x, edge_index, W1, a_src1, a_dst1, b1, W2, a_src2, a_dst2, b2):
    trace = bool(int(os.environ.get("GAT_TRACE", "0")))
    LAST_EXEC_TIMES.clear()
    LAST_RUNS.clear()
    LAST_TRACES.clear()

    sched, streams, w1r, xT_cores, w2r = _prepare(
        x, edge_index, W1, a_src1, a_dst1, W2, a_src2, a_dst2)

    # ---- P1
    nc1 = build_p1()
    in1 = [dict(xT=xT_cores[k], w1r=w1r) for k in range(NC_CORES)]
    r1 = _run(nc1, in1, trace)
    t1_full = np.concatenate([r1[k]["t1s"] for k in range(NC_CORES)], axis=0)
    ad1s = [r1[k]["ad1s"] for k in range(NC_CORES)]

    # ---- P2
    b1v = np.asarray(b1, np.float32).reshape(1, D1)
    nc2 = build_edge_layer(sched, 1)
    in2 = [dict(tfull=t1_full, adt=ad1s[k], gidx=streams[k]["gidx"],
                doT=streams[k]["doT16"], do16=streams[k]["do16"],
                w2r=w2r, b1v=b1v)
           for k in range(NC_CORES)]
    r2 = _run(nc2, in2, trace)
    t2_full = np.concatenate([r2[k]["t2s"] for k in range(NC_CORES)], axis=0)
    ad2s = [r2[k]["ad2s"] for k in range(NC_CORES)]

    # ---- P3
    b2v = np.asarray(b2, np.float32).reshape(1, D2)
    nc3 = build_edge_layer(sched, 2)
    in3 = [dict(tfull=t2_full, adt=ad2s[k], gidx=streams[k]["gidx"],
                doT=streams[k]["doT16"], do16=streams[k]["do16"], b2v=b2v)
           for k in range(NC_CORES)]
    r3 = _run(nc3, in3, trace)
    out = np.concatenate([r3[k]["res"] for k in range(NC_CORES)], axis=0)
    return out[:N].astype(np.float32)
